# revision 10
# baseline (speedup 1.0000x reference)
import sys
for p in ('/opt/trn_rl_repo', '/opt/pypackages'):
    if p not in sys.path:
        sys.path.insert(0, p)
import numpy as np
from concourse import bass, bacc, tile, mybir
from concourse import bass_utils

B, C, T, K = 4, 64, 4096, 4
NCORES = 8
OS = T // NCORES          # 512: per-core token slice (phase-1 o-slice == phase-2 t-slice)
BC = B * C                # 256
f32 = mybir.dt.float32
f16 = mybir.dt.float16
u32 = mybir.dt.uint32

_cache = {}
LAST_EXEC_NS = []
LAST_TRACE_DIRS = []


def _run(nc, in_maps):
    r = bass_utils.run_bass_kernel_spmd(nc, in_maps, core_ids=list(range(NCORES)))
    LAST_EXEC_NS.append(getattr(r, "exec_time_ns", None))
    it = getattr(r, "instructions_and_trace", None)
    if it is not None:
        LAST_TRACE_DIRS.append(it[1])
    return r


def _build_l1():
    """Phase 1, SPMD core j: single kt-sweep computing q,k (fp16 split-3) and v
    (fp16) o-slices with grouped stationary operands; emits qn hi/lo f16,
    k hi/lo f16 and the conv-folded value tables u."""
    nc = bacc.Bacc("TRN2", target_bir_lowering=False, debug=False, num_devices=NCORES)
    XH = nc.dram_tensor("xh", [128, T // 128, BC], f16, kind="ExternalInput").ap()
    XL = nc.dram_tensor("xl", [128, T // 128, BC], f16, kind="ExternalInput").ap()
    WQH = nc.dram_tensor("wqh", [128, T // 128, OS], f16, kind="ExternalInput").ap()
    WQL = nc.dram_tensor("wql", [128, T // 128, OS], f16, kind="ExternalInput").ap()
    WKH = nc.dram_tensor("wkh", [128, T // 128, OS], f16, kind="ExternalInput").ap()
    WKL = nc.dram_tensor("wkl", [128, T // 128, OS], f16, kind="ExternalInput").ap()
    WV = nc.dram_tensor("wv", [128, T // 128, OS], f16, kind="ExternalInput").ap()
    CW = nc.dram_tensor("cw", [2 * C, K * C], f16, kind="ExternalInput").ap()
    QNH = nc.dram_tensor("qnh_o", [BC, OS], f16, kind="ExternalOutput").ap()
    QNL = nc.dram_tensor("qnl_o", [BC, OS], f16, kind="ExternalOutput").ap()
    KH = nc.dram_tensor("kh_o", [BC, OS], f16, kind="ExternalOutput").ap()
    KL = nc.dram_tensor("kl_o", [BC, OS], f16, kind="ExternalOutput").ap()
    UO = nc.dram_tensor("u_o", [B, K, 4, 128, C], f16, kind="ExternalOutput").ap()

    NKT = T // 128  # 32 contraction tiles
    NHF = 4
    H = NKT // NHF  # 8 kt per weight chunk

    with tile.TileContext(nc) as tc:
        with tc.tile_pool(name="xp", bufs=1) as xp, \
             tc.tile_pool(name="wp", bufs=2) as wp, \
             tc.tile_pool(name="sp", bufs=2) as sp, \
             tc.tile_pool(name="cp", bufs=1) as cp, \
             tc.tile_pool(name="pp", bufs=1, space="PSUM") as pp, \
             tc.tile_pool(name="pu", bufs=1, space="PSUM") as pu:
            xh = xp.tile([128, NKT, BC], f16, tag="xh")
            xl = xp.tile([128, NKT, BC], f16, tag="xl")
            nc.sync.dma_start(out=xh[:, :, :], in_=XH[:, :, :])
            nc.sync.dma_start(out=xl[:, :, :], in_=XL[:, :, :])
            cw = cp.tile([2 * C, K * C], f16, tag="cw")
            nc.sync.dma_start(out=cw[:, :], in_=CW[:, :])
            ones_r = cp.tile([128, 1], f32, tag="ones_r")   # reduce lhsT
            nc.vector.memset(ones_r[:, :], 1.0)
            ones_b = cp.tile([1, C], f32, tag="ones_b")   # broadcast lhsT
            nc.vector.memset(ones_b[:, :], 1.0)

            qacc, kacc, vacc = [], [], []
            for mt in range(2):
                qacc.append(pp.tile([128, OS], f32, tag=f"qa{mt}", name=f"qa{mt}"))
                kacc.append(pp.tile([128, OS], f32, tag=f"ka{mt}", name=f"ka{mt}"))
                vacc.append(pp.tile([128, OS], f32, tag=f"va{mt}", name=f"va{mt}"))

            for hf in range(NHF):
                wqh = wp.tile([128, H, OS], f16, tag="wqh")
                wql = wp.tile([128, H, OS], f16, tag="wql")
                wkh = wp.tile([128, H, OS], f16, tag="wkh")
                wkl = wp.tile([128, H, OS], f16, tag="wkl")
                wv = wp.tile([128, H, OS], f16, tag="wv")
                sl = slice(hf * H, (hf + 1) * H)
                nc.sync.dma_start(out=wqh[:, :, :], in_=WQH[:, sl, :])
                nc.sync.dma_start(out=wql[:, :, :], in_=WQL[:, sl, :])
                nc.sync.dma_start(out=wkh[:, :, :], in_=WKH[:, sl, :])
                nc.sync.dma_start(out=wkl[:, :, :], in_=WKL[:, sl, :])
                nc.sync.dma_start(out=wv[:, :, :], in_=WV[:, sl, :])
                for t in range(H):
                    kt = hf * H + t
                    first = (kt == 0)
                    last = (kt == NKT - 1)
                    for mt in range(2):
                        lh = xh[:, kt, mt * 128:(mt + 1) * 128]
                        ll = xl[:, kt, mt * 128:(mt + 1) * 128]
                        # lh-stationary group
                        nc.tensor.matmul(out=qacc[mt][:, :], lhsT=lh, rhs=wqh[:, t, :],
                                         start=first, stop=False)
                        nc.tensor.matmul(out=qacc[mt][:, :], lhsT=lh, rhs=wql[:, t, :],
                                         start=False, stop=False)
                        nc.tensor.matmul(out=kacc[mt][:, :], lhsT=lh, rhs=wkh[:, t, :],
                                         start=first, stop=False)
                        nc.tensor.matmul(out=kacc[mt][:, :], lhsT=lh, rhs=wkl[:, t, :],
                                         start=False, stop=False)
                        nc.tensor.matmul(out=vacc[mt][:, :], lhsT=lh, rhs=wv[:, t, :],
                                         start=first, stop=last)
                        # ll-stationary group
                        nc.tensor.matmul(out=qacc[mt][:, :], lhsT=ll, rhs=wqh[:, t, :],
                                         start=False, stop=last)
                        nc.tensor.matmul(out=kacc[mt][:, :], lhsT=ll, rhs=wkh[:, t, :],
                                         start=False, stop=last)

            # ---- k: split hi/lo f16 and store ----
            for mt in range(2):
                kf = sp.tile([128, OS], f32, tag=f"kf{mt}")
                nc.scalar.copy(out=kf[:, :], in_=kacc[mt][:, :])
                kh16 = sp.tile([128, OS], f16, tag=f"kh{mt}")
                nc.scalar.copy(out=kh16[:, :], in_=kf[:, :])
                kh32 = sp.tile([128, OS], f32, tag=f"kh32{mt}")
                nc.scalar.copy(out=kh32[:, :], in_=kh16[:, :])
                kl16 = sp.tile([128, OS], f16, tag=f"kl{mt}")
                nc.vector.tensor_sub(out=kl16[:, :], in0=kf[:, :], in1=kh32[:, :])
                nc.sync.dma_start(out=KH[mt * 128:(mt + 1) * 128, :], in_=kh16[:, :])
                nc.sync.dma_start(out=KL[mt * 128:(mt + 1) * 128, :], in_=kl16[:, :])

            # ---- v to f16; u tables: u[b,k] = (v_b^T @ cw_k) as 4 M-tiles ----
            vsb = {}
            for mt in range(2):
                v16 = sp.tile([128, OS], f16, tag=f"vsb{mt}")
                nc.scalar.copy(out=v16[:, :], in_=vacc[mt][:, :])
                vsb[mt] = v16
            for b in range(B):
                off = (b % 2) * C
                vt = vsb[b // 2][off:off + C, :]  # [64, 512] f16
                for k in range(K):
                    u16 = sp.tile([128, 4, C], f16, tag="u16")
                    for m in range(4):
                        pt = pu.tile([128, C], f32, tag="pu")
                        nc.tensor.matmul(out=pt[:, :],
                                         lhsT=vt[:, m * 128:(m + 1) * 128],
                                         rhs=cw[off:off + C, k * C:(k + 1) * C],
                                         start=True, stop=True)
                        nc.scalar.copy(out=u16[:, m, :], in_=pt[:, :])
                    for m in range(4):
                        nc.sync.dma_start(out=UO[b, k, m], in_=u16[:, m, :])

            # ---- qn = q / ||q||_col, emitted as hi/lo f16 ----
            for b in range(B):
                off = (b % 2) * C
                q_b = qacc[b // 2][off:off + C, :]  # [64, 512] f32 psum
                qf = sp.tile([128, OS], f32, tag="qf")
                nc.scalar.copy(out=qf[off:off + C, :], in_=q_b)
                sq = sp.tile([128, OS], f32, tag="sq")
                nc.scalar.square(out=sq[off:off + C, :], in_=qf[off:off + C, :])
                pscr = pu.tile([128, OS], f32, tag="pscr", name="pscr")
                nc.tensor.matmul(out=pscr[0:1, :], lhsT=ones_r[off:off + C, :],
                                 rhs=sq[off:off + C, :], start=True, stop=True)
                nrm = sp.tile([1, OS], f32, tag="nrm")
                nc.scalar.sqrt(out=nrm[:, :], in_=pscr[0:1, :])
                rcp = sp.tile([1, OS], f32, tag="rcp")
                nc.vector.reciprocal(out=rcp[:, :], in_=nrm[:, :])
                pb = pu.tile([128, OS], f32, tag="pscr", name="pb")
                nc.tensor.matmul(out=pb[off:off + C, :], lhsT=ones_b[:, :],
                                 rhs=rcp[:, :], start=True, stop=True)
                bc = sp.tile([128, OS], f32, tag="bc")
                nc.scalar.copy(out=bc[off:off + C, :], in_=pb[off:off + C, :])
                qn = sp.tile([128, OS], f32, tag="qn")
                nc.vector.tensor_mul(out=qn[off:off + C, :], in0=qf[off:off + C, :],
                                     in1=bc[off:off + C, :])
                qh16 = sp.tile([128, OS], f16, tag="qh16")
                nc.scalar.copy(out=qh16[off:off + C, :], in_=qn[off:off + C, :])
                qh32 = sp.tile([128, OS], f32, tag="qh32")
                nc.scalar.copy(out=qh32[off:off + C, :], in_=qh16[off:off + C, :])
                ql16 = sp.tile([128, OS], f16, tag="ql16")
                nc.vector.tensor_sub(out=ql16[off:off + C, :],
                                     in0=qn[off:off + C, :], in1=qh32[off:off + C, :])
                nc.sync.dma_start(out=QNH[b * C:(b + 1) * C, :], in_=qh16[off:off + C, :])
                nc.sync.dma_start(out=QNL[b * C:(b + 1) * C, :], in_=ql16[off:off + C, :])
    nc.compile()
    return nc


def _build_l2():
    """Phase 2, SPMD core j: rows t in [j*512,(j+1)*512) for all batches.
    Exact sim via fp16 split-3 matmuls in 4 psum chunks of 1024; per-chunk
    exact top-8 via max/max_index on the f32 copy; merge 32 candidates via
    max8 + find_index + one-hot decode; gather-sum u tables -> yT;
    partial out = yT^T @ WoT slice."""
    nc = bacc.Bacc("TRN2", target_bir_lowering=False, debug=False, num_devices=NCORES)
    QNH = nc.dram_tensor("qnh", [BC, T], f16, kind="ExternalInput").ap()
    QNL = nc.dram_tensor("qnl", [BC, T], f16, kind="ExternalInput").ap()
    KHJ = nc.dram_tensor("khj", [BC, OS], f16, kind="ExternalInput").ap()
    KLJ = nc.dram_tensor("klj", [BC, OS], f16, kind="ExternalInput").ap()
    WOT = nc.dram_tensor("wot", [OS, T], f16, kind="ExternalInput").ap()
    IOTA = nc.dram_tensor("iota32", [128, 32], u32, kind="ExternalInput").ap()
    UT = [[nc.dram_tensor(f"ut{b}_{k}", [T, C], f16, kind="ExternalInput").ap()
           for k in range(K)] for b in range(B)]
    OUT = nc.dram_tensor("out_o", [2, 128, T], f32, kind="ExternalOutput").ap()

    NCH = 4          # sim chunks per block
    CW_ = T // NCH   # 1024 columns per chunk

    with tile.TileContext(nc) as tc:
        with tc.tile_pool(name="qp", bufs=1) as qp, \
             tc.tile_pool(name="wp", bufs=1) as wp, \
             tc.tile_pool(name="sp", bufs=3) as sp, \
             tc.tile_pool(name="simp", bufs=3) as simp, \
             tc.tile_pool(name="yp", bufs=1) as yp, \
             tc.tile_pool(name="pp", bufs=2, space="PSUM") as pp, \
             tc.tile_pool(name="po", bufs=2, space="PSUM") as po:
            qh, ql = {}, {}
            for mt in range(2):
                h16 = qp.tile([128, T], f16, tag=f"qh{mt}")
                nc.sync.dma_start(out=h16[:, :], in_=QNH[mt * 128:(mt + 1) * 128, :])
                l16 = qp.tile([128, T], f16, tag=f"ql{mt}")
                nc.sync.dma_start(out=l16[:, :], in_=QNL[mt * 128:(mt + 1) * 128, :])
                qh[mt], ql[mt] = h16, l16
            kh, kl = {}, {}
            for mt in range(2):
                h16 = qp.tile([128, OS], f16, tag=f"kh{mt}")
                nc.sync.dma_start(out=h16[:, :], in_=KHJ[mt * 128:(mt + 1) * 128, :])
                l16 = qp.tile([128, OS], f16, tag=f"kl{mt}")
                nc.sync.dma_start(out=l16[:, :], in_=KLJ[mt * 128:(mt + 1) * 128, :])
                kh[mt], kl[mt] = h16, l16
            wot = {}
            for kt in range(4):
                w = wp.tile([128, T], f16, tag=f"wot{kt}")
                nc.sync.dma_start(out=w[:, :], in_=WOT[kt * 128:(kt + 1) * 128, :])
                wot[kt] = w
            iota = qp.tile([128, 32], u32, tag="iota")
            nc.sync.dma_start(out=iota[:, :], in_=IOTA[:, :])

            ytp = {}  # (pair, kt) -> [128, 128] f16 lhsT tiles for the out matmul
            for pair in range(2):
                for kt in range(4):
                    yt_tile = yp.tile([128, 128], f16, tag=f"yt{pair}{kt}")
                    ytp[(pair, kt)] = yt_tile

            for b in range(B):
                khb = kh[b // 2][(b % 2) * C:(b % 2) * C + C, :]   # [64, 512] f16
                klb = kl[b // 2][(b % 2) * C:(b % 2) * C + C, :]
                qhb = qh[b // 2][(b % 2) * C:(b % 2) * C + C, :]   # [64, 4096] f16
                qlb = ql[b // 2][(b % 2) * C:(b % 2) * C + C, :]
                for i in range(4):  # 128-token blocks of this core's slice
                    lh = khb[:, i * 128:(i + 1) * 128]
                    ll = klb[:, i * 128:(i + 1) * 128]
                    m32 = sp.tile([128, 32], f32, tag="m32")
                    simf = simp.tile([128, T], f32, tag="simf")
                    for ch in range(NCH):
                        ps = pp.tile([128, CW_], f32, tag="ps")
                        for half in range(2):
                            c0 = ch * CW_ + half * 512
                            rh = qhb[:, c0:c0 + 512]
                            rl = qlb[:, c0:c0 + 512]
                            po_s = ps[:, half * 512:(half + 1) * 512]
                            nc.tensor.matmul(out=po_s, lhsT=lh, rhs=rh,
                                             start=True, stop=False)
                            nc.tensor.matmul(out=po_s, lhsT=ll, rhs=rh,
                                             start=False, stop=False)
                            nc.tensor.matmul(out=po_s, lhsT=lh, rhs=rl,
                                             start=False, stop=True)
                        nc.scalar.copy(out=simf[:, ch * CW_:(ch + 1) * CW_], in_=ps[:, :])
                        nc.vector.max(out=m32[:, ch * 8:(ch + 1) * 8],
                                      in_=simf[:, ch * CW_:(ch + 1) * CW_])
                    # merge: global top-8 values, then one full-row index scan
                    g8 = sp.tile([128, 8], f32, tag="g8")
                    nc.vector.max(out=g8[:, :], in_=m32[:, :])
                    idx8 = sp.tile([128, 8], u32, tag="idx8")
                    nc.vector.max_index(out=idx8[:, :], in_max=g8[:, :],
                                        in_values=simf[:, :])
                    gth = sp.tile([128, K, C], f16, tag="gth")
                    for k in range(K):
                        nc.gpsimd.indirect_dma_start(
                            out=gth[:, k, :], out_offset=None,
                            in_=UT[b][k][:, :],
                            in_offset=bass.IndirectOffsetOnAxis(ap=idx8[:, k:k + 1], axis=0))
                    t0 = sp.tile([128, C], f16, tag="t0")
                    t1 = sp.tile([128, C], f16, tag="t1")
                    nc.gpsimd.tensor_add(out=t0[:, :], in0=gth[:, 0, :], in1=gth[:, 1, :])
                    nc.gpsimd.tensor_add(out=t1[:, :], in0=gth[:, 2, :], in1=gth[:, 3, :])
                    dst = ytp[(b // 2, i)][:, (b % 2) * C:(b % 2) * C + C]
                    nc.gpsimd.tensor_add(out=dst, in0=t0[:, :], in1=t1[:, :])

            # partial out: for batch pair, out[128(2b,c), T] = sum_kt ytp^T @ wot
            for pair in range(2):
                ob = simp.tile([128, T], f32, tag="ob")
                for ch in range(NCH):
                    ps = po.tile([128, CW_], f32, tag="po")
                    for half in range(2):
                        c0 = ch * CW_ + half * 512
                        po_s = ps[:, half * 512:(half + 1) * 512]
                        for kt in range(4):
                            nc.tensor.matmul(out=po_s, lhsT=ytp[(pair, kt)][:, :],
                                             rhs=wot[kt][:, c0:c0 + 512],
                                             start=(kt == 0), stop=(kt == 3))
                    nc.scalar.copy(out=ob[:, ch * CW_:(ch + 1) * CW_], in_=ps[:, :])
                nc.sync.dma_start(out=OUT[pair], in_=ob[:, :])
    nc.compile()
    return nc


def _split16(a):
    h = a.astype(np.float16)
    l = (a - h.astype(np.float32)).astype(np.float16)
    return h, l


def kernel(x, Wq, Wk, Wv, Wo, conv_w, conv_b):
    x = np.asarray(x, np.float32)
    Wq = np.asarray(Wq, np.float32); Wk = np.asarray(Wk, np.float32)
    Wv = np.asarray(Wv, np.float32); Wo = np.asarray(Wo, np.float32)
    conv_w = np.asarray(conv_w, np.float32); conv_b = np.asarray(conv_b, np.float32)

    LAST_EXEC_NS.clear(); LAST_TRACE_DIRS.clear()
    if "l1" not in _cache:
        _cache["l1"] = _build_l1()
    if "l2" not in _cache:
        _cache["l2"] = _build_l2()

    def _sw(a):
        # [T, W] -> [128, T//128, W] with [p, kt, w] = a[kt*128+p, w]
        return np.ascontiguousarray(a.reshape(T // 128, 128, -1).transpose(1, 0, 2))

    xT = np.ascontiguousarray(x.transpose(2, 0, 1).reshape(T, BC))  # [t, b*64+c]
    xh, xl = _split16(xT)
    xh, xl = _sw(xh), _sw(xl)
    WqT, WkT = Wq.T, Wk.T
    WvT16 = np.ascontiguousarray(Wv.T).astype(np.float16)
    cw1 = np.ascontiguousarray(conv_w.transpose(1, 2, 0).reshape(C, K * C)).astype(np.float16)
    cw = np.concatenate([cw1, cw1], axis=0)
    # cw[ci, k*64+co] = conv_w[co, ci, k]

    in_maps = []
    for j in range(NCORES):
        sl = slice(j * OS, (j + 1) * OS)
        wqh, wql = _split16(np.ascontiguousarray(WqT[:, sl]))
        wkh, wkl = _split16(np.ascontiguousarray(WkT[:, sl]))
        in_maps.append({"xh": xh, "xl": xl,
                        "wqh": _sw(wqh), "wql": _sw(wql),
                        "wkh": _sw(wkh), "wkl": _sw(wkl),
                        "wv": _sw(np.ascontiguousarray(WvT16[:, sl])), "cw": cw})
    r1 = _run(_cache["l1"], in_maps)

    qnh = np.concatenate([r1.results[j]["qnh_o"] for j in range(NCORES)], axis=1)
    qnl = np.concatenate([r1.results[j]["qnl_o"] for j in range(NCORES)], axis=1)
    ut = {}
    for b in range(B):
        for k in range(K):
            ut[(b, k)] = np.ascontiguousarray(np.concatenate(
                [r1.results[j]["u_o"][b, k].reshape(OS, C) for j in range(NCORES)], axis=0))

    iota32 = np.broadcast_to(np.arange(32, dtype=np.uint32), (128, 32)).copy()
    in_maps2 = []
    for j in range(NCORES):
        m = {"qnh": qnh, "qnl": qnl,
             "khj": r1.results[j]["kh_o"], "klj": r1.results[j]["kl_o"],
             "wot": np.ascontiguousarray(Wo.T[j * OS:(j + 1) * OS, :]).astype(np.float16),
             "iota32": iota32}
        for b in range(B):
            for k in range(K):
                m[f"ut{b}_{k}"] = ut[(b, k)]
        in_maps2.append(m)
    r2 = _run(_cache["l2"], in_maps2)

    out = np.zeros((B, C, T), np.float32)
    for j in range(NCORES):
        oo = r2.results[j]["out_o"]  # [2, 128, T]
        for b in range(B):
            out[b] += oo[b // 2, (b % 2) * C:(b % 2) * C + C, :]
    bias = conv_b[:, None] * Wo.sum(axis=1)[None, :]  # [64, 4096]
    out += bias[None, :, :]
    return out


# revision 14
# speedup vs baseline: 1.0003x; 1.0003x over previous
import sys
for p in ('/opt/trn_rl_repo', '/opt/pypackages'):
    if p not in sys.path:
        sys.path.insert(0, p)
import numpy as np
from concourse import bass, bacc, tile, mybir
from concourse import bass_utils

B, C, T, K = 4, 64, 4096, 4
NCORES = 8
OS = T // NCORES          # 512: per-core token slice (phase-1 o-slice == phase-2 t-slice)
BC = B * C                # 256
f32 = mybir.dt.float32
f16 = mybir.dt.float16
u32 = mybir.dt.uint32

_cache = {}
LAST_EXEC_NS = []
LAST_TRACE_DIRS = []


def _run(nc, in_maps):
    r = bass_utils.run_bass_kernel_spmd(nc, in_maps, core_ids=list(range(NCORES)))
    LAST_EXEC_NS.append(getattr(r, "exec_time_ns", None))
    it = getattr(r, "instructions_and_trace", None)
    if it is not None:
        LAST_TRACE_DIRS.append(it[1])
    return r


def _build_l1():
    """Phase 1, SPMD core j: single kt-sweep computing q,k (fp16 split-3) and v
    (fp16) o-slices with grouped stationary operands; emits qn hi/lo f16,
    k hi/lo f16 and the conv-folded value tables u."""
    nc = bacc.Bacc("TRN2", target_bir_lowering=False, debug=False, num_devices=NCORES)
    XH = nc.dram_tensor("xh", [128, T // 128, BC], f16, kind="ExternalInput").ap()
    XL = nc.dram_tensor("xl", [128, T // 128, BC], f16, kind="ExternalInput").ap()
    WQH = nc.dram_tensor("wqh", [128, T // 128, OS], f16, kind="ExternalInput").ap()
    WQL = nc.dram_tensor("wql", [128, T // 128, OS], f16, kind="ExternalInput").ap()
    WKH = nc.dram_tensor("wkh", [128, T // 128, OS], f16, kind="ExternalInput").ap()
    WKL = nc.dram_tensor("wkl", [128, T // 128, OS], f16, kind="ExternalInput").ap()
    WV = nc.dram_tensor("wv", [128, T // 128, OS], f16, kind="ExternalInput").ap()
    CW = nc.dram_tensor("cw", [2 * C, K * C], f16, kind="ExternalInput").ap()
    QNH = nc.dram_tensor("qnh_o", [BC, OS], f16, kind="ExternalOutput").ap()
    QNL = nc.dram_tensor("qnl_o", [BC, OS], f16, kind="ExternalOutput").ap()
    KH = nc.dram_tensor("kh_o", [BC, OS], f16, kind="ExternalOutput").ap()
    KL = nc.dram_tensor("kl_o", [BC, OS], f16, kind="ExternalOutput").ap()
    UO = nc.dram_tensor("u_o", [B, K, 4, 128, C], f16, kind="ExternalOutput").ap()

    NKT = T // 128  # 32 contraction tiles
    NHF = 4
    H = NKT // NHF  # 8 kt per weight chunk

    with tile.TileContext(nc) as tc:
        with tc.tile_pool(name="xp", bufs=1) as xp, \
             tc.tile_pool(name="wp", bufs=2) as wp, \
             tc.tile_pool(name="sp", bufs=2) as sp, \
             tc.tile_pool(name="cp", bufs=1) as cp, \
             tc.tile_pool(name="pp", bufs=1, space="PSUM") as pp, \
             tc.tile_pool(name="pu", bufs=1, space="PSUM") as pu:
            xh = xp.tile([128, NKT, BC], f16, tag="xh")
            xl = xp.tile([128, NKT, BC], f16, tag="xl")
            for hf in range(NHF):
                sl = slice(hf * H, (hf + 1) * H)
                nc.sync.dma_start(out=xh[:, sl, :], in_=XH[:, sl, :])
                nc.sync.dma_start(out=xl[:, sl, :], in_=XL[:, sl, :])
            cw = cp.tile([2 * C, K * C], f16, tag="cw")
            nc.sync.dma_start(out=cw[:, :], in_=CW[:, :])
            ones_r = cp.tile([128, 1], f32, tag="ones_r")   # reduce lhsT
            nc.vector.memset(ones_r[:, :], 1.0)
            ones_b = cp.tile([1, C], f32, tag="ones_b")   # broadcast lhsT
            nc.vector.memset(ones_b[:, :], 1.0)

            qacc, kacc, vacc = [], [], []
            for mt in range(2):
                qacc.append(pp.tile([128, OS], f32, tag=f"qa{mt}", name=f"qa{mt}"))
                kacc.append(pp.tile([128, OS], f32, tag=f"ka{mt}", name=f"ka{mt}"))
                vacc.append(pp.tile([128, OS], f32, tag=f"va{mt}", name=f"va{mt}"))

            for hf in range(NHF):
                wqh = wp.tile([128, H, OS], f16, tag="wqh")
                wql = wp.tile([128, H, OS], f16, tag="wql")
                wkh = wp.tile([128, H, OS], f16, tag="wkh")
                wkl = wp.tile([128, H, OS], f16, tag="wkl")
                wv = wp.tile([128, H, OS], f16, tag="wv")
                sl = slice(hf * H, (hf + 1) * H)
                nc.sync.dma_start(out=wqh[:, :, :], in_=WQH[:, sl, :])
                nc.sync.dma_start(out=wql[:, :, :], in_=WQL[:, sl, :])
                nc.sync.dma_start(out=wkh[:, :, :], in_=WKH[:, sl, :])
                nc.sync.dma_start(out=wkl[:, :, :], in_=WKL[:, sl, :])
                nc.sync.dma_start(out=wv[:, :, :], in_=WV[:, sl, :])
                for t in range(H):
                    kt = hf * H + t
                    first = (kt == 0)
                    last = (kt == NKT - 1)
                    for mt in range(2):
                        lh = xh[:, kt, mt * 128:(mt + 1) * 128]
                        ll = xl[:, kt, mt * 128:(mt + 1) * 128]
                        # lh-stationary group
                        nc.tensor.matmul(out=qacc[mt][:, :], lhsT=lh, rhs=wqh[:, t, :],
                                         start=first, stop=False)
                        nc.tensor.matmul(out=qacc[mt][:, :], lhsT=lh, rhs=wql[:, t, :],
                                         start=False, stop=False)
                        nc.tensor.matmul(out=kacc[mt][:, :], lhsT=lh, rhs=wkh[:, t, :],
                                         start=first, stop=False)
                        nc.tensor.matmul(out=kacc[mt][:, :], lhsT=lh, rhs=wkl[:, t, :],
                                         start=False, stop=False)
                        nc.tensor.matmul(out=vacc[mt][:, :], lhsT=lh, rhs=wv[:, t, :],
                                         start=first, stop=last)
                        # ll-stationary group
                        nc.tensor.matmul(out=qacc[mt][:, :], lhsT=ll, rhs=wqh[:, t, :],
                                         start=False, stop=last)
                        nc.tensor.matmul(out=kacc[mt][:, :], lhsT=ll, rhs=wkh[:, t, :],
                                         start=False, stop=last)

            # ---- k: split hi/lo f16 and store ----
            for mt in range(2):
                kf = sp.tile([128, OS], f32, tag=f"kf{mt}")
                nc.scalar.copy(out=kf[:, :], in_=kacc[mt][:, :])
                kh16 = sp.tile([128, OS], f16, tag=f"kh{mt}")
                nc.scalar.copy(out=kh16[:, :], in_=kf[:, :])
                kh32 = sp.tile([128, OS], f32, tag=f"kh32{mt}")
                nc.scalar.copy(out=kh32[:, :], in_=kh16[:, :])
                kl16 = sp.tile([128, OS], f16, tag=f"kl{mt}")
                nc.vector.tensor_sub(out=kl16[:, :], in0=kf[:, :], in1=kh32[:, :])
                nc.sync.dma_start(out=KH[mt * 128:(mt + 1) * 128, :], in_=kh16[:, :])
                nc.sync.dma_start(out=KL[mt * 128:(mt + 1) * 128, :], in_=kl16[:, :])

            # ---- v to f16; u tables: u[b,k] = (v_b^T @ cw_k) as 4 M-tiles ----
            vsb = {}
            for mt in range(2):
                v16 = sp.tile([128, OS], f16, tag=f"vsb{mt}")
                nc.scalar.copy(out=v16[:, :], in_=vacc[mt][:, :])
                vsb[mt] = v16
            for b in range(B):
                off = (b % 2) * C
                vt = vsb[b // 2][off:off + C, :]  # [64, 512] f16
                for k in range(K):
                    u16 = sp.tile([128, 4, C], f16, tag="u16")
                    for m in range(4):
                        pt = pu.tile([128, C], f32, tag="pu")
                        nc.tensor.matmul(out=pt[:, :],
                                         lhsT=vt[:, m * 128:(m + 1) * 128],
                                         rhs=cw[off:off + C, k * C:(k + 1) * C],
                                         start=True, stop=True)
                        nc.scalar.copy(out=u16[:, m, :], in_=pt[:, :])
                    for m in range(4):
                        nc.sync.dma_start(out=UO[b, k, m], in_=u16[:, m, :])

            # ---- qn = q / ||q||_col, emitted as hi/lo f16 ----
            for b in range(B):
                off = (b % 2) * C
                q_b = qacc[b // 2][off:off + C, :]  # [64, 512] f32 psum
                qf = sp.tile([128, OS], f32, tag="qf")
                nc.scalar.copy(out=qf[off:off + C, :], in_=q_b)
                sq = sp.tile([128, OS], f32, tag="sq")
                nc.scalar.square(out=sq[off:off + C, :], in_=qf[off:off + C, :])
                pscr = pu.tile([128, OS], f32, tag="pscr", name="pscr")
                nc.tensor.matmul(out=pscr[0:1, :], lhsT=ones_r[off:off + C, :],
                                 rhs=sq[off:off + C, :], start=True, stop=True)
                nrm = sp.tile([1, OS], f32, tag="nrm")
                nc.scalar.sqrt(out=nrm[:, :], in_=pscr[0:1, :])
                rcp = sp.tile([1, OS], f32, tag="rcp")
                nc.vector.reciprocal(out=rcp[:, :], in_=nrm[:, :])
                pb = pu.tile([128, OS], f32, tag="pscr", name="pb")
                nc.tensor.matmul(out=pb[off:off + C, :], lhsT=ones_b[:, :],
                                 rhs=rcp[:, :], start=True, stop=True)
                bc = sp.tile([128, OS], f32, tag="bc")
                nc.scalar.copy(out=bc[off:off + C, :], in_=pb[off:off + C, :])
                qn = sp.tile([128, OS], f32, tag="qn")
                nc.vector.tensor_mul(out=qn[off:off + C, :], in0=qf[off:off + C, :],
                                     in1=bc[off:off + C, :])
                qh16 = sp.tile([128, OS], f16, tag="qh16")
                nc.scalar.copy(out=qh16[off:off + C, :], in_=qn[off:off + C, :])
                qh32 = sp.tile([128, OS], f32, tag="qh32")
                nc.scalar.copy(out=qh32[off:off + C, :], in_=qh16[off:off + C, :])
                ql16 = sp.tile([128, OS], f16, tag="ql16")
                nc.vector.tensor_sub(out=ql16[off:off + C, :],
                                     in0=qn[off:off + C, :], in1=qh32[off:off + C, :])
                nc.sync.dma_start(out=QNH[b * C:(b + 1) * C, :], in_=qh16[off:off + C, :])
                nc.sync.dma_start(out=QNL[b * C:(b + 1) * C, :], in_=ql16[off:off + C, :])
    nc.compile()
    return nc


def _build_l2():
    """Phase 2, SPMD core j: rows t in [j*512,(j+1)*512) for all batches.
    Exact sim via fp16 split-3 matmuls in 4 psum chunks of 1024; per-chunk
    exact top-8 via max/max_index on the f32 copy; merge 32 candidates via
    max8 + find_index + one-hot decode; gather-sum u tables -> yT;
    partial out = yT^T @ WoT slice."""
    nc = bacc.Bacc("TRN2", target_bir_lowering=False, debug=False, num_devices=NCORES)
    QNH = nc.dram_tensor("qnh", [BC, T], f16, kind="ExternalInput").ap()
    QNL = nc.dram_tensor("qnl", [BC, T], f16, kind="ExternalInput").ap()
    KHJ = nc.dram_tensor("khj", [BC, OS], f16, kind="ExternalInput").ap()
    KLJ = nc.dram_tensor("klj", [BC, OS], f16, kind="ExternalInput").ap()
    WOT = nc.dram_tensor("wot", [OS, T], f16, kind="ExternalInput").ap()
    IOTA = nc.dram_tensor("iota32", [128, 32], u32, kind="ExternalInput").ap()
    UT = [[nc.dram_tensor(f"ut{b}_{k}", [T, C], f16, kind="ExternalInput").ap()
           for k in range(K)] for b in range(B)]
    OUT = nc.dram_tensor("out_o", [2, 128, T], f32, kind="ExternalOutput").ap()

    NCH = 4          # sim chunks per block
    CW_ = T // NCH   # 1024 columns per chunk

    with tile.TileContext(nc) as tc:
        with tc.tile_pool(name="qp", bufs=1) as qp, \
             tc.tile_pool(name="wp", bufs=1) as wp, \
             tc.tile_pool(name="sp", bufs=3) as sp, \
             tc.tile_pool(name="simp", bufs=4) as simp, \
             tc.tile_pool(name="yp", bufs=1) as yp, \
             tc.tile_pool(name="pp", bufs=2, space="PSUM") as pp, \
             tc.tile_pool(name="po", bufs=2, space="PSUM") as po:
            qh, ql = {}, {}
            for mt in range(2):
                h16 = qp.tile([128, T], f16, tag=f"qh{mt}")
                nc.sync.dma_start(out=h16[:, :], in_=QNH[mt * 128:(mt + 1) * 128, :])
                l16 = qp.tile([128, T], f16, tag=f"ql{mt}")
                nc.sync.dma_start(out=l16[:, :], in_=QNL[mt * 128:(mt + 1) * 128, :])
                qh[mt], ql[mt] = h16, l16
            kh, kl = {}, {}
            for mt in range(2):
                h16 = qp.tile([128, OS], f16, tag=f"kh{mt}")
                nc.sync.dma_start(out=h16[:, :], in_=KHJ[mt * 128:(mt + 1) * 128, :])
                l16 = qp.tile([128, OS], f16, tag=f"kl{mt}")
                nc.sync.dma_start(out=l16[:, :], in_=KLJ[mt * 128:(mt + 1) * 128, :])
                kh[mt], kl[mt] = h16, l16
            wot = {}
            for kt in range(4):
                w = wp.tile([128, T], f16, tag=f"wot{kt}")
                nc.sync.dma_start(out=w[:, :], in_=WOT[kt * 128:(kt + 1) * 128, :])
                wot[kt] = w
            iota = qp.tile([128, 32], u32, tag="iota")
            nc.sync.dma_start(out=iota[:, :], in_=IOTA[:, :])

            ytp = {}  # (pair, kt) -> [128, 128] f16 lhsT tiles for the out matmul
            for pair in range(2):
                for kt in range(4):
                    yt_tile = yp.tile([128, 128], f16, tag=f"yt{pair}{kt}")
                    ytp[(pair, kt)] = yt_tile

            for b in range(B):
                khb = kh[b // 2][(b % 2) * C:(b % 2) * C + C, :]   # [64, 512] f16
                klb = kl[b // 2][(b % 2) * C:(b % 2) * C + C, :]
                qhb = qh[b // 2][(b % 2) * C:(b % 2) * C + C, :]   # [64, 4096] f16
                qlb = ql[b // 2][(b % 2) * C:(b % 2) * C + C, :]
                for i in range(4):  # 128-token blocks of this core's slice
                    lh = khb[:, i * 128:(i + 1) * 128]
                    ll = klb[:, i * 128:(i + 1) * 128]
                    m32 = sp.tile([128, 32], f32, tag="m32")
                    simf = simp.tile([128, T], f32, tag="simf")
                    for ch in range(NCH):
                        ps = pp.tile([128, CW_], f32, tag="ps")
                        for half in range(2):
                            c0 = ch * CW_ + half * 512
                            rh = qhb[:, c0:c0 + 512]
                            rl = qlb[:, c0:c0 + 512]
                            po_s = ps[:, half * 512:(half + 1) * 512]
                            nc.tensor.matmul(out=po_s, lhsT=lh, rhs=rh,
                                             start=True, stop=False)
                            nc.tensor.matmul(out=po_s, lhsT=ll, rhs=rh,
                                             start=False, stop=False)
                            nc.tensor.matmul(out=po_s, lhsT=lh, rhs=rl,
                                             start=False, stop=True)
                        nc.scalar.copy(out=simf[:, ch * CW_:(ch + 1) * CW_], in_=ps[:, :])
                        nc.vector.max(out=m32[:, ch * 8:(ch + 1) * 8],
                                      in_=simf[:, ch * CW_:(ch + 1) * CW_])
                    # merge: global top-8 values, then one full-row index scan
                    g8 = sp.tile([128, 8], f32, tag="g8")
                    nc.vector.max(out=g8[:, :], in_=m32[:, :])
                    idx8 = sp.tile([128, 8], u32, tag="idx8")
                    nc.vector.max_index(out=idx8[:, :], in_max=g8[:, :],
                                        in_values=simf[:, :])
                    gth = sp.tile([128, K, C], f16, tag="gth")
                    for k in range(K):
                        nc.gpsimd.indirect_dma_start(
                            out=gth[:, k, :], out_offset=None,
                            in_=UT[b][k][:, :],
                            in_offset=bass.IndirectOffsetOnAxis(ap=idx8[:, k:k + 1], axis=0))
                    t0 = sp.tile([128, C], f16, tag="t0")
                    t1 = sp.tile([128, C], f16, tag="t1")
                    nc.gpsimd.tensor_add(out=t0[:, :], in0=gth[:, 0, :], in1=gth[:, 1, :])
                    nc.gpsimd.tensor_add(out=t1[:, :], in0=gth[:, 2, :], in1=gth[:, 3, :])
                    dst = ytp[(b // 2, i)][:, (b % 2) * C:(b % 2) * C + C]
                    nc.gpsimd.tensor_add(out=dst, in0=t0[:, :], in1=t1[:, :])

            # partial out: for batch pair, out[128(2b,c), T] = sum_kt ytp^T @ wot
            for pair in range(2):
                ob = simp.tile([128, T], f32, tag="ob")
                for ch in range(NCH):
                    ps = po.tile([128, CW_], f32, tag="po")
                    for half in range(2):
                        c0 = ch * CW_ + half * 512
                        po_s = ps[:, half * 512:(half + 1) * 512]
                        for kt in range(4):
                            nc.tensor.matmul(out=po_s, lhsT=ytp[(pair, kt)][:, :],
                                             rhs=wot[kt][:, c0:c0 + 512],
                                             start=(kt == 0), stop=(kt == 3))
                    nc.scalar.copy(out=ob[:, ch * CW_:(ch + 1) * CW_], in_=ps[:, :])
                nc.sync.dma_start(out=OUT[pair], in_=ob[:, :])
    nc.compile()
    return nc


def _split16(a):
    h = a.astype(np.float16)
    l = (a - h.astype(np.float32)).astype(np.float16)
    return h, l


def kernel(x, Wq, Wk, Wv, Wo, conv_w, conv_b):
    x = np.asarray(x, np.float32)
    Wq = np.asarray(Wq, np.float32); Wk = np.asarray(Wk, np.float32)
    Wv = np.asarray(Wv, np.float32); Wo = np.asarray(Wo, np.float32)
    conv_w = np.asarray(conv_w, np.float32); conv_b = np.asarray(conv_b, np.float32)

    LAST_EXEC_NS.clear(); LAST_TRACE_DIRS.clear()
    if "l1" not in _cache:
        _cache["l1"] = _build_l1()
    if "l2" not in _cache:
        _cache["l2"] = _build_l2()

    def _sw(a):
        # [T, W] -> [128, T//128, W] with [p, kt, w] = a[kt*128+p, w]
        return np.ascontiguousarray(a.reshape(T // 128, 128, -1).transpose(1, 0, 2))

    xT = np.ascontiguousarray(x.transpose(2, 0, 1).reshape(T, BC))  # [t, b*64+c]
    xh, xl = _split16(xT)
    xh, xl = _sw(xh), _sw(xl)
    WqT, WkT = Wq.T, Wk.T
    WvT16 = np.ascontiguousarray(Wv.T).astype(np.float16)
    cw1 = np.ascontiguousarray(conv_w.transpose(1, 2, 0).reshape(C, K * C)).astype(np.float16)
    cw = np.concatenate([cw1, cw1], axis=0)
    # cw[ci, k*64+co] = conv_w[co, ci, k]

    in_maps = []
    for j in range(NCORES):
        sl = slice(j * OS, (j + 1) * OS)
        wqh, wql = _split16(np.ascontiguousarray(WqT[:, sl]))
        wkh, wkl = _split16(np.ascontiguousarray(WkT[:, sl]))
        in_maps.append({"xh": xh, "xl": xl,
                        "wqh": _sw(wqh), "wql": _sw(wql),
                        "wkh": _sw(wkh), "wkl": _sw(wkl),
                        "wv": _sw(np.ascontiguousarray(WvT16[:, sl])), "cw": cw})
    r1 = _run(_cache["l1"], in_maps)

    qnh = np.concatenate([r1.results[j]["qnh_o"] for j in range(NCORES)], axis=1)
    qnl = np.concatenate([r1.results[j]["qnl_o"] for j in range(NCORES)], axis=1)
    ut = {}
    for b in range(B):
        for k in range(K):
            ut[(b, k)] = np.ascontiguousarray(np.concatenate(
                [r1.results[j]["u_o"][b, k].reshape(OS, C) for j in range(NCORES)], axis=0))

    iota32 = np.broadcast_to(np.arange(32, dtype=np.uint32), (128, 32)).copy()
    in_maps2 = []
    for j in range(NCORES):
        m = {"qnh": qnh, "qnl": qnl,
             "khj": r1.results[j]["kh_o"], "klj": r1.results[j]["kl_o"],
             "wot": np.ascontiguousarray(Wo.T[j * OS:(j + 1) * OS, :]).astype(np.float16),
             "iota32": iota32}
        for b in range(B):
            for k in range(K):
                m[f"ut{b}_{k}"] = ut[(b, k)]
        in_maps2.append(m)
    r2 = _run(_cache["l2"], in_maps2)

    out = np.zeros((B, C, T), np.float32)
    for j in range(NCORES):
        oo = r2.results[j]["out_o"]  # [2, 128, T]
        for b in range(B):
            out[b] += oo[b // 2, (b % 2) * C:(b % 2) * C + C, :]
    bias = conv_b[:, None] * Wo.sum(axis=1)[None, :]  # [64, 4096]
    out += bias[None, :, :]
    return out


# revision 19
# speedup vs baseline: 1.0682x; 1.0679x over previous
import sys
for p in ('/opt/trn_rl_repo', '/opt/pypackages'):
    if p not in sys.path:
        sys.path.insert(0, p)
import numpy as np
from concourse import bass, bacc, tile, mybir
from concourse import bass_utils

B, C, T, K = 4, 64, 4096, 4
NCORES = 8
OS = T // NCORES          # 512: per-core token slice (phase-1 o-slice == phase-2 t-slice)
BC = B * C                # 256
f32 = mybir.dt.float32
f16 = mybir.dt.float16
u32 = mybir.dt.uint32

_cache = {}
LAST_EXEC_NS = []
LAST_TRACE_DIRS = []


def _run(nc, in_maps):
    r = bass_utils.run_bass_kernel_spmd(nc, in_maps, core_ids=list(range(NCORES)))
    LAST_EXEC_NS.append(getattr(r, "exec_time_ns", None))
    it = getattr(r, "instructions_and_trace", None)
    if it is not None:
        LAST_TRACE_DIRS.append(it[1])
    return r


def _build_l1():
    """Phase 1, SPMD core j: single kt-sweep computing q,k (fp16 split-3) and v
    (fp16) o-slices with grouped stationary operands; emits qn hi/lo f16,
    k hi/lo f16 and the conv-folded value tables u."""
    nc = bacc.Bacc("TRN2", target_bir_lowering=False, debug=False, num_devices=NCORES)
    XH = nc.dram_tensor("xh", [128, T // 128, BC], f16, kind="ExternalInput").ap()
    XL = nc.dram_tensor("xl", [128, T // 128, BC], f16, kind="ExternalInput").ap()
    WQH = nc.dram_tensor("wqh", [128, T // 128, OS], f16, kind="ExternalInput").ap()
    WQL = nc.dram_tensor("wql", [128, T // 128, OS], f16, kind="ExternalInput").ap()
    WKH = nc.dram_tensor("wkh", [128, T // 128, OS], f16, kind="ExternalInput").ap()
    WKL = nc.dram_tensor("wkl", [128, T // 128, OS], f16, kind="ExternalInput").ap()
    WV = nc.dram_tensor("wv", [128, T // 128, OS], f16, kind="ExternalInput").ap()
    CW = nc.dram_tensor("cw", [2 * C, K * C], f16, kind="ExternalInput").ap()
    QNH = nc.dram_tensor("qnh_o", [BC, OS], f16, kind="ExternalOutput").ap()
    QNL = nc.dram_tensor("qnl_o", [BC, OS], f16, kind="ExternalOutput").ap()
    KH = nc.dram_tensor("kh_o", [BC, OS], f16, kind="ExternalOutput").ap()
    KL = nc.dram_tensor("kl_o", [BC, OS], f16, kind="ExternalOutput").ap()
    UO = nc.dram_tensor("u_o", [B, K, 4, 128, C], f16, kind="ExternalOutput").ap()

    NKT = T // 128  # 32 contraction tiles
    NHF = 4
    H = NKT // NHF  # 8 kt per weight chunk

    with tile.TileContext(nc) as tc:
        with tc.tile_pool(name="xp", bufs=1) as xp, \
             tc.tile_pool(name="wp", bufs=2) as wp, \
             tc.tile_pool(name="sp", bufs=2) as sp, \
             tc.tile_pool(name="cp", bufs=1) as cp, \
             tc.tile_pool(name="pp", bufs=1, space="PSUM") as pp, \
             tc.tile_pool(name="pu", bufs=1, space="PSUM") as pu:
            xh = xp.tile([128, NKT, BC], f16, tag="xh")
            xl = xp.tile([128, NKT, BC], f16, tag="xl")
            cw = cp.tile([2 * C, K * C], f16, tag="cw")
            nc.sync.dma_start(out=cw[:, :], in_=CW[:, :])
            ones_r = cp.tile([128, 1], f32, tag="ones_r")   # reduce lhsT
            nc.vector.memset(ones_r[:, :], 1.0)
            ones_b = cp.tile([1, C], f32, tag="ones_b")   # broadcast lhsT
            nc.vector.memset(ones_b[:, :], 1.0)

            qacc, kacc, vacc = [], [], []
            for mt in range(2):
                qacc.append(pp.tile([128, OS], f32, tag=f"qa{mt}", name=f"qa{mt}"))
                kacc.append(pp.tile([128, OS], f32, tag=f"ka{mt}", name=f"ka{mt}"))
                vacc.append(pp.tile([128, OS], f32, tag=f"va{mt}", name=f"va{mt}"))

            for hf in range(NHF):
                slx = slice(hf * H, (hf + 1) * H)
                nc.sync.dma_start(out=xh[:, slx, :], in_=XH[:, slx, :])
                nc.sync.dma_start(out=xl[:, slx, :], in_=XL[:, slx, :])
                wqh = wp.tile([128, H, OS], f16, tag="wqh")
                wql = wp.tile([128, H, OS], f16, tag="wql")
                wkh = wp.tile([128, H, OS], f16, tag="wkh")
                wkl = wp.tile([128, H, OS], f16, tag="wkl")
                wv = wp.tile([128, H, OS], f16, tag="wv")
                sl = slice(hf * H, (hf + 1) * H)
                nc.sync.dma_start(out=wqh[:, :, :], in_=WQH[:, sl, :])
                nc.sync.dma_start(out=wql[:, :, :], in_=WQL[:, sl, :])
                nc.sync.dma_start(out=wkh[:, :, :], in_=WKH[:, sl, :])
                nc.sync.dma_start(out=wkl[:, :, :], in_=WKL[:, sl, :])
                nc.sync.dma_start(out=wv[:, :, :], in_=WV[:, sl, :])
                for t in range(H):
                    kt = hf * H + t
                    first = (kt == 0)
                    last = (kt == NKT - 1)
                    for mt in range(2):
                        lh = xh[:, kt, mt * 128:(mt + 1) * 128]
                        ll = xl[:, kt, mt * 128:(mt + 1) * 128]
                        # lh-stationary group
                        nc.tensor.matmul(out=qacc[mt][:, :], lhsT=lh, rhs=wqh[:, t, :],
                                         start=first, stop=False)
                        nc.tensor.matmul(out=qacc[mt][:, :], lhsT=lh, rhs=wql[:, t, :],
                                         start=False, stop=False)
                        nc.tensor.matmul(out=kacc[mt][:, :], lhsT=lh, rhs=wkh[:, t, :],
                                         start=first, stop=False)
                        nc.tensor.matmul(out=kacc[mt][:, :], lhsT=lh, rhs=wkl[:, t, :],
                                         start=False, stop=False)
                        nc.tensor.matmul(out=vacc[mt][:, :], lhsT=lh, rhs=wv[:, t, :],
                                         start=first, stop=last)
                        # ll-stationary group
                        nc.tensor.matmul(out=qacc[mt][:, :], lhsT=ll, rhs=wqh[:, t, :],
                                         start=False, stop=last)
                        nc.tensor.matmul(out=kacc[mt][:, :], lhsT=ll, rhs=wkh[:, t, :],
                                         start=False, stop=last)

            # ---- k: split hi/lo f16 and store ----
            for mt in range(2):
                kf = sp.tile([128, OS], f32, tag=f"kf{mt}")
                nc.scalar.copy(out=kf[:, :], in_=kacc[mt][:, :])
                kh16 = sp.tile([128, OS], f16, tag=f"kh{mt}")
                nc.scalar.copy(out=kh16[:, :], in_=kf[:, :])
                kh32 = sp.tile([128, OS], f32, tag=f"kh32{mt}")
                nc.scalar.copy(out=kh32[:, :], in_=kh16[:, :])
                kl16 = sp.tile([128, OS], f16, tag=f"kl{mt}")
                nc.vector.tensor_sub(out=kl16[:, :], in0=kf[:, :], in1=kh32[:, :])
                nc.sync.dma_start(out=KH[mt * 128:(mt + 1) * 128, :], in_=kh16[:, :])
                nc.sync.dma_start(out=KL[mt * 128:(mt + 1) * 128, :], in_=kl16[:, :])

            # ---- v to f16; u tables: u[b,k] = (v_b^T @ cw_k) as 4 M-tiles ----
            vsb = {}
            for mt in range(2):
                v16 = sp.tile([128, OS], f16, tag=f"vsb{mt}")
                nc.scalar.copy(out=v16[:, :], in_=vacc[mt][:, :])
                vsb[mt] = v16
            for b in range(B):
                off = (b % 2) * C
                vt = vsb[b // 2][off:off + C, :]  # [64, 512] f16
                for k in range(K):
                    u16 = sp.tile([128, 4, C], f16, tag="u16")
                    for m in range(4):
                        pt = pu.tile([128, C], f32, tag="pu")
                        nc.tensor.matmul(out=pt[:, :],
                                         lhsT=vt[:, m * 128:(m + 1) * 128],
                                         rhs=cw[off:off + C, k * C:(k + 1) * C],
                                         start=True, stop=True)
                        nc.scalar.copy(out=u16[:, m, :], in_=pt[:, :])
                    for m in range(4):
                        nc.sync.dma_start(out=UO[b, k, m], in_=u16[:, m, :])

            # ---- qn = q / ||q||_col, emitted as hi/lo f16 ----
            for b in range(B):
                off = (b % 2) * C
                q_b = qacc[b // 2][off:off + C, :]  # [64, 512] f32 psum
                qf = sp.tile([128, OS], f32, tag="qf")
                nc.scalar.copy(out=qf[off:off + C, :], in_=q_b)
                sq = sp.tile([128, OS], f32, tag="sq")
                nc.scalar.square(out=sq[off:off + C, :], in_=qf[off:off + C, :])
                pscr = pu.tile([128, OS], f32, tag="pscr", name="pscr")
                nc.tensor.matmul(out=pscr[0:1, :], lhsT=ones_r[off:off + C, :],
                                 rhs=sq[off:off + C, :], start=True, stop=True)
                nrm = sp.tile([1, OS], f32, tag="nrm")
                nc.scalar.sqrt(out=nrm[:, :], in_=pscr[0:1, :])
                rcp = sp.tile([1, OS], f32, tag="rcp")
                nc.vector.reciprocal(out=rcp[:, :], in_=nrm[:, :])
                pb = pu.tile([128, OS], f32, tag="pscr", name="pb")
                nc.tensor.matmul(out=pb[off:off + C, :], lhsT=ones_b[:, :],
                                 rhs=rcp[:, :], start=True, stop=True)
                bc = sp.tile([128, OS], f32, tag="bc")
                nc.scalar.copy(out=bc[off:off + C, :], in_=pb[off:off + C, :])
                qn = sp.tile([128, OS], f32, tag="qn")
                nc.vector.tensor_mul(out=qn[off:off + C, :], in0=qf[off:off + C, :],
                                     in1=bc[off:off + C, :])
                qh16 = sp.tile([128, OS], f16, tag="qh16")
                nc.scalar.copy(out=qh16[off:off + C, :], in_=qn[off:off + C, :])
                qh32 = sp.tile([128, OS], f32, tag="qh32")
                nc.scalar.copy(out=qh32[off:off + C, :], in_=qh16[off:off + C, :])
                ql16 = sp.tile([128, OS], f16, tag="ql16")
                nc.vector.tensor_sub(out=ql16[off:off + C, :],
                                     in0=qn[off:off + C, :], in1=qh32[off:off + C, :])
                nc.sync.dma_start(out=QNH[b * C:(b + 1) * C, :], in_=qh16[off:off + C, :])
                nc.sync.dma_start(out=QNL[b * C:(b + 1) * C, :], in_=ql16[off:off + C, :])
    nc.compile()
    return nc


def _build_l2():
    """Phase 2, SPMD core j: rows t in [j*512,(j+1)*512) for all batches.
    Exact sim via fp16 split-3 matmuls in 4 psum chunks of 1024; per-chunk
    exact top-8 via max/max_index on the f32 copy; merge 32 candidates via
    max8 + find_index + one-hot decode; gather-sum u tables -> yT;
    partial out = yT^T @ WoT slice."""
    nc = bacc.Bacc("TRN2", target_bir_lowering=False, debug=False, num_devices=NCORES)
    QNH = nc.dram_tensor("qnh", [BC, T], f16, kind="ExternalInput").ap()
    QNL = nc.dram_tensor("qnl", [BC, T], f16, kind="ExternalInput").ap()
    KHJ = nc.dram_tensor("khj", [BC, OS], f16, kind="ExternalInput").ap()
    KLJ = nc.dram_tensor("klj", [BC, OS], f16, kind="ExternalInput").ap()
    WOT = nc.dram_tensor("wot", [OS, T], f16, kind="ExternalInput").ap()
    IOTA = nc.dram_tensor("iota32", [128, 32], u32, kind="ExternalInput").ap()
    UT = [[nc.dram_tensor(f"ut{b}_{k}", [T, C], f16, kind="ExternalInput").ap()
           for k in range(K)] for b in range(B)]
    OUT = nc.dram_tensor("out_o", [2, 128, T], f32, kind="ExternalOutput").ap()

    NCH = 4          # sim chunks per block
    CW_ = T // NCH   # 1024 columns per chunk

    with tile.TileContext(nc) as tc:
        with tc.tile_pool(name="qp", bufs=1) as qp, \
             tc.tile_pool(name="wp", bufs=1) as wp, \
             tc.tile_pool(name="sp", bufs=3) as sp, \
             tc.tile_pool(name="simp", bufs=4) as simp, \
             tc.tile_pool(name="yp", bufs=1) as yp, \
             tc.tile_pool(name="pp", bufs=4, space="PSUM") as pp, \
             tc.tile_pool(name="po", bufs=2, space="PSUM") as po:
            kh, kl = {}, {}
            for mt in range(2):
                h16 = qp.tile([128, OS], f16, tag=f"kh{mt}")
                nc.sync.dma_start(out=h16[:, :], in_=KHJ[mt * 128:(mt + 1) * 128, :])
                l16 = qp.tile([128, OS], f16, tag=f"kl{mt}")
                nc.sync.dma_start(out=l16[:, :], in_=KLJ[mt * 128:(mt + 1) * 128, :])
                kh[mt], kl[mt] = h16, l16
            qh, ql = {}, {}
            for mt in range(2):
                h16 = qp.tile([128, T], f16, tag=f"qh{mt}")
                l16 = qp.tile([128, T], f16, tag=f"ql{mt}")
                for cc in range(4):
                    cs = slice(cc * 1024, (cc + 1) * 1024)
                    nc.sync.dma_start(out=h16[:, cs], in_=QNH[mt * 128:(mt + 1) * 128, cs])
                    nc.sync.dma_start(out=l16[:, cs], in_=QNL[mt * 128:(mt + 1) * 128, cs])
                qh[mt], ql[mt] = h16, l16
            wot = {}
            iota = qp.tile([128, 32], u32, tag="iota")
            nc.sync.dma_start(out=iota[:, :], in_=IOTA[:, :])

            ytp = {}  # (pair, kt) -> [128, 128] f16 lhsT tiles for the out matmul
            for pair in range(2):
                for kt in range(4):
                    yt_tile = yp.tile([128, 128], f16, tag=f"yt{pair}{kt}")
                    ytp[(pair, kt)] = yt_tile

            for b in range(B):
                khb = kh[b // 2][(b % 2) * C:(b % 2) * C + C, :]   # [64, 512] f16
                klb = kl[b // 2][(b % 2) * C:(b % 2) * C + C, :]
                qhb = qh[b // 2][(b % 2) * C:(b % 2) * C + C, :]   # [64, 4096] f16
                qlb = ql[b // 2][(b % 2) * C:(b % 2) * C + C, :]
                for i in range(4):  # 128-token blocks of this core's slice
                    lh = khb[:, i * 128:(i + 1) * 128]
                    ll = klb[:, i * 128:(i + 1) * 128]
                    m32 = sp.tile([128, 32], f32, tag="m32")
                    simf = simp.tile([128, T], f32, tag="simf")
                    for ch in range(NCH):
                        for half in range(2):
                            c0 = ch * CW_ + half * 512
                            rh = qhb[:, c0:c0 + 512]
                            rl = qlb[:, c0:c0 + 512]
                            ps = pp.tile([128, 512], f32, tag="ps")
                            nc.tensor.matmul(out=ps[:, :], lhsT=lh, rhs=rh,
                                             start=True, stop=False)
                            nc.tensor.matmul(out=ps[:, :], lhsT=ll, rhs=rh,
                                             start=False, stop=False)
                            nc.tensor.matmul(out=ps[:, :], lhsT=lh, rhs=rl,
                                             start=False, stop=True)
                            nc.scalar.copy(out=simf[:, c0:c0 + 512], in_=ps[:, :])
                        nc.vector.max(out=m32[:, ch * 8:(ch + 1) * 8],
                                      in_=simf[:, ch * CW_:(ch + 1) * CW_])
                    # merge: global top-8 values, then one full-row index scan
                    g8 = sp.tile([128, 8], f32, tag="g8")
                    nc.vector.max(out=g8[:, :], in_=m32[:, :])
                    idx8 = sp.tile([128, 8], u32, tag="idx8")
                    nc.vector.max_index(out=idx8[:, :], in_max=g8[:, :],
                                        in_values=simf[:, :])
                    gth = sp.tile([128, K, C], f16, tag="gth")
                    for k in range(K):
                        nc.gpsimd.indirect_dma_start(
                            out=gth[:, k, :], out_offset=None,
                            in_=UT[b][k][:, :],
                            in_offset=bass.IndirectOffsetOnAxis(ap=idx8[:, k:k + 1], axis=0))
                    t0 = sp.tile([128, C], f16, tag="t0")
                    t1 = sp.tile([128, C], f16, tag="t1")
                    nc.gpsimd.tensor_add(out=t0[:, :], in0=gth[:, 0, :], in1=gth[:, 1, :])
                    nc.gpsimd.tensor_add(out=t1[:, :], in0=gth[:, 2, :], in1=gth[:, 3, :])
                    dst = ytp[(b // 2, i)][:, (b % 2) * C:(b % 2) * C + C]
                    nc.gpsimd.tensor_add(out=dst, in0=t0[:, :], in1=t1[:, :])

                # emit the pair's out-projection as soon as both batches done
                if b % 2 == 1:
                    pair = b // 2
                    if not wot:
                        for kt in range(4):
                            w = wp.tile([128, T], f16, tag=f"wot{kt}", name=f"wot{kt}")
                            nc.sync.dma_start(out=w[:, :], in_=WOT[kt * 128:(kt + 1) * 128, :])
                            wot[kt] = w
                    ob = simp.tile([128, T], f32, tag="ob")
                    for ch in range(NCH):
                        ps = po.tile([128, CW_], f32, tag="po")
                        for half in range(2):
                            c0 = ch * CW_ + half * 512
                            po_s = ps[:, half * 512:(half + 1) * 512]
                            for kt in range(4):
                                nc.tensor.matmul(out=po_s, lhsT=ytp[(pair, kt)][:, :],
                                                 rhs=wot[kt][:, c0:c0 + 512],
                                                 start=(kt == 0), stop=(kt == 3))
                        nc.scalar.copy(out=ob[:, ch * CW_:(ch + 1) * CW_], in_=ps[:, :])
                        nc.sync.dma_start(out=OUT[pair, :, ch * CW_:(ch + 1) * CW_],
                                          in_=ob[:, ch * CW_:(ch + 1) * CW_])


    nc.compile()
    return nc


def _split16(a):
    h = a.astype(np.float16)
    l = (a - h.astype(np.float32)).astype(np.float16)
    return h, l


def kernel(x, Wq, Wk, Wv, Wo, conv_w, conv_b):
    x = np.asarray(x, np.float32)
    Wq = np.asarray(Wq, np.float32); Wk = np.asarray(Wk, np.float32)
    Wv = np.asarray(Wv, np.float32); Wo = np.asarray(Wo, np.float32)
    conv_w = np.asarray(conv_w, np.float32); conv_b = np.asarray(conv_b, np.float32)

    LAST_EXEC_NS.clear(); LAST_TRACE_DIRS.clear()
    if "l1" not in _cache:
        _cache["l1"] = _build_l1()
    if "l2" not in _cache:
        _cache["l2"] = _build_l2()

    def _sw(a):
        # [T, W] -> [128, T//128, W] with [p, kt, w] = a[kt*128+p, w]
        return np.ascontiguousarray(a.reshape(T // 128, 128, -1).transpose(1, 0, 2))

    xT = np.ascontiguousarray(x.transpose(2, 0, 1).reshape(T, BC))  # [t, b*64+c]
    xh, xl = _split16(xT)
    xh, xl = _sw(xh), _sw(xl)
    WqT, WkT = Wq.T, Wk.T
    WvT16 = np.ascontiguousarray(Wv.T).astype(np.float16)
    cw1 = np.ascontiguousarray(conv_w.transpose(1, 2, 0).reshape(C, K * C)).astype(np.float16)
    cw = np.concatenate([cw1, cw1], axis=0)
    # cw[ci, k*64+co] = conv_w[co, ci, k]

    in_maps = []
    for j in range(NCORES):
        sl = slice(j * OS, (j + 1) * OS)
        wqh, wql = _split16(np.ascontiguousarray(WqT[:, sl]))
        wkh, wkl = _split16(np.ascontiguousarray(WkT[:, sl]))
        in_maps.append({"xh": xh, "xl": xl,
                        "wqh": _sw(wqh), "wql": _sw(wql),
                        "wkh": _sw(wkh), "wkl": _sw(wkl),
                        "wv": _sw(np.ascontiguousarray(WvT16[:, sl])), "cw": cw})
    r1 = _run(_cache["l1"], in_maps)

    qnh = np.concatenate([r1.results[j]["qnh_o"] for j in range(NCORES)], axis=1)
    qnl = np.concatenate([r1.results[j]["qnl_o"] for j in range(NCORES)], axis=1)
    ut = {}
    for b in range(B):
        for k in range(K):
            ut[(b, k)] = np.ascontiguousarray(np.concatenate(
                [r1.results[j]["u_o"][b, k].reshape(OS, C) for j in range(NCORES)], axis=0))

    iota32 = np.broadcast_to(np.arange(32, dtype=np.uint32), (128, 32)).copy()
    in_maps2 = []
    for j in range(NCORES):
        m = {"qnh": qnh, "qnl": qnl,
             "khj": r1.results[j]["kh_o"], "klj": r1.results[j]["kl_o"],
             "wot": np.ascontiguousarray(Wo.T[j * OS:(j + 1) * OS, :]).astype(np.float16),
             "iota32": iota32}
        for b in range(B):
            for k in range(K):
                m[f"ut{b}_{k}"] = ut[(b, k)]
        in_maps2.append(m)
    r2 = _run(_cache["l2"], in_maps2)

    out = np.zeros((B, C, T), np.float32)
    for j in range(NCORES):
        oo = r2.results[j]["out_o"]  # [2, 128, T]
        for b in range(B):
            out[b] += oo[b // 2, (b % 2) * C:(b % 2) * C + C, :]
    bias = conv_b[:, None] * Wo.sum(axis=1)[None, :]  # [64, 4096]
    out += bias[None, :, :]
    return out


# revision 27
# speedup vs baseline: 1.0699x; 1.0016x over previous
import sys
for p in ('/opt/trn_rl_repo', '/opt/pypackages'):
    if p not in sys.path:
        sys.path.insert(0, p)
import numpy as np
from concourse import bass, bacc, tile, mybir
from concourse import bass_utils

B, C, T, K = 4, 64, 4096, 4
NCORES = 8
OS = T // NCORES          # 512: per-core token slice (phase-1 o-slice == phase-2 t-slice)
BC = B * C                # 256
f32 = mybir.dt.float32
f16 = mybir.dt.float16
u32 = mybir.dt.uint32

_cache = {}
LAST_EXEC_NS = []
LAST_TRACE_DIRS = []


def _run(nc, in_maps):
    r = bass_utils.run_bass_kernel_spmd(nc, in_maps, core_ids=list(range(NCORES)))
    LAST_EXEC_NS.append(getattr(r, "exec_time_ns", None))
    it = getattr(r, "instructions_and_trace", None)
    if it is not None:
        LAST_TRACE_DIRS.append(it[1])
    return r


def _build_l1():
    """Phase 1, SPMD core j: single kt-sweep computing q,k (fp16 split-3) and v
    (fp16) o-slices with grouped stationary operands; emits qn hi/lo f16,
    k hi/lo f16 and the conv-folded value tables u."""
    nc = bacc.Bacc("TRN2", target_bir_lowering=False, debug=False, num_devices=NCORES)
    XH = nc.dram_tensor("xh", [128, T // 128, BC], f16, kind="ExternalInput").ap()
    XL = nc.dram_tensor("xl", [128, T // 128, BC], f16, kind="ExternalInput").ap()
    WQH = nc.dram_tensor("wqh", [128, T // 128, OS], f16, kind="ExternalInput").ap()
    WQL = nc.dram_tensor("wql", [128, T // 128, OS], f16, kind="ExternalInput").ap()
    WKH = nc.dram_tensor("wkh", [128, T // 128, OS], f16, kind="ExternalInput").ap()
    WKL = nc.dram_tensor("wkl", [128, T // 128, OS], f16, kind="ExternalInput").ap()
    WV = nc.dram_tensor("wv", [128, T // 128, OS], f16, kind="ExternalInput").ap()
    CW = nc.dram_tensor("cw", [2 * C, K * C], f16, kind="ExternalInput").ap()
    QNH = nc.dram_tensor("qnh_o", [BC, OS], f16, kind="ExternalOutput").ap()
    QNL = nc.dram_tensor("qnl_o", [BC, OS], f16, kind="ExternalOutput").ap()
    KH = nc.dram_tensor("kh_o", [BC, OS], f16, kind="ExternalOutput").ap()
    KL = nc.dram_tensor("kl_o", [BC, OS], f16, kind="ExternalOutput").ap()
    UO = nc.dram_tensor("u_o", [B, K, 4, 128, C], f16, kind="ExternalOutput").ap()

    NKT = T // 128  # 32 contraction tiles
    NHF = 4
    H = NKT // NHF  # 8 kt per weight chunk

    with tile.TileContext(nc) as tc:
        with tc.tile_pool(name="xp", bufs=1) as xp, \
             tc.tile_pool(name="wp", bufs=2) as wp, \
             tc.tile_pool(name="sp", bufs=2) as sp, \
             tc.tile_pool(name="cp", bufs=1) as cp, \
             tc.tile_pool(name="pp", bufs=1, space="PSUM") as pp, \
             tc.tile_pool(name="pu", bufs=1, space="PSUM") as pu:
            xh = xp.tile([128, NKT, BC], f16, tag="xh")
            xl = xp.tile([128, NKT, BC], f16, tag="xl")
            cw = cp.tile([2 * C, K * C], f16, tag="cw")
            nc.sync.dma_start(out=cw[:, :], in_=CW[:, :])
            ones_r = cp.tile([128, 1], f32, tag="ones_r")   # reduce lhsT
            nc.vector.memset(ones_r[:, :], 1.0)
            ones_b = cp.tile([1, C], f32, tag="ones_b")   # broadcast lhsT
            nc.vector.memset(ones_b[:, :], 1.0)

            wu1 = cp.tile([128, 512], f16, tag="wu1")
            nc.vector.memset(wu1[:, :], 0.0)
            for _w in range(6):
                pw = pu.tile([128, OS], f32, tag="pscr", name=f"warm{_w}")
                nc.tensor.matmul(out=pw[:, :], lhsT=wu1[:, :128], rhs=wu1[:, :],
                                 start=True, stop=True)

            qacc, kacc, vacc = [], [], []
            for mt in range(2):
                qacc.append(pp.tile([128, OS], f32, tag=f"qa{mt}", name=f"qa{mt}"))
                kacc.append(pp.tile([128, OS], f32, tag=f"ka{mt}", name=f"ka{mt}"))
                vacc.append(pp.tile([128, OS], f32, tag=f"va{mt}", name=f"va{mt}"))

            for hf in range(NHF):
                slx = slice(hf * H, (hf + 1) * H)
                nc.sync.dma_start(out=xh[:, slx, :], in_=XH[:, slx, :])
                nc.sync.dma_start(out=xl[:, slx, :], in_=XL[:, slx, :])
                wqh = wp.tile([128, H, OS], f16, tag="wqh")
                wql = wp.tile([128, H, OS], f16, tag="wql")
                wkh = wp.tile([128, H, OS], f16, tag="wkh")
                wkl = wp.tile([128, H, OS], f16, tag="wkl")
                wv = wp.tile([128, H, OS], f16, tag="wv")
                sl = slice(hf * H, (hf + 1) * H)
                nc.sync.dma_start(out=wqh[:, :, :], in_=WQH[:, sl, :])
                nc.sync.dma_start(out=wql[:, :, :], in_=WQL[:, sl, :])
                nc.sync.dma_start(out=wkh[:, :, :], in_=WKH[:, sl, :])
                nc.sync.dma_start(out=wkl[:, :, :], in_=WKL[:, sl, :])
                nc.sync.dma_start(out=wv[:, :, :], in_=WV[:, sl, :])
                for t in range(H):
                    kt = hf * H + t
                    first = (kt == 0)
                    last = (kt == NKT - 1)
                    for mt in range(2):
                        lh = xh[:, kt, mt * 128:(mt + 1) * 128]
                        ll = xl[:, kt, mt * 128:(mt + 1) * 128]
                        # lh-stationary group
                        nc.tensor.matmul(out=qacc[mt][:, :], lhsT=lh, rhs=wqh[:, t, :],
                                         start=first, stop=False)
                        nc.tensor.matmul(out=qacc[mt][:, :], lhsT=lh, rhs=wql[:, t, :],
                                         start=False, stop=False)
                        nc.tensor.matmul(out=kacc[mt][:, :], lhsT=lh, rhs=wkh[:, t, :],
                                         start=first, stop=False)
                        nc.tensor.matmul(out=kacc[mt][:, :], lhsT=lh, rhs=wkl[:, t, :],
                                         start=False, stop=False)
                        nc.tensor.matmul(out=vacc[mt][:, :], lhsT=lh, rhs=wv[:, t, :],
                                         start=first, stop=last)
                        # ll-stationary group
                        nc.tensor.matmul(out=qacc[mt][:, :], lhsT=ll, rhs=wqh[:, t, :],
                                         start=False, stop=last)
                        nc.tensor.matmul(out=kacc[mt][:, :], lhsT=ll, rhs=wkh[:, t, :],
                                         start=False, stop=last)

            # ---- k: split hi/lo f16 and store ----
            for mt in range(2):
                kf = sp.tile([128, OS], f32, tag=f"kf{mt}")
                nc.scalar.copy(out=kf[:, :], in_=kacc[mt][:, :])
                kh16 = sp.tile([128, OS], f16, tag=f"kh{mt}")
                nc.scalar.copy(out=kh16[:, :], in_=kf[:, :])
                kh32 = sp.tile([128, OS], f32, tag=f"kh32{mt}")
                nc.scalar.copy(out=kh32[:, :], in_=kh16[:, :])
                kl16 = sp.tile([128, OS], f16, tag=f"kl{mt}")
                nc.vector.tensor_sub(out=kl16[:, :], in0=kf[:, :], in1=kh32[:, :])
                nc.sync.dma_start(out=KH[mt * 128:(mt + 1) * 128, :], in_=kh16[:, :])
                nc.sync.dma_start(out=KL[mt * 128:(mt + 1) * 128, :], in_=kl16[:, :])

            # ---- v to f16; u tables: u[b,k] = (v_b^T @ cw_k) as 4 M-tiles ----
            vsb = {}
            for mt in range(2):
                v16 = sp.tile([128, OS], f16, tag=f"vsb{mt}")
                nc.scalar.copy(out=v16[:, :], in_=vacc[mt][:, :])
                vsb[mt] = v16
            for b in range(B):
                off = (b % 2) * C
                vt = vsb[b // 2][off:off + C, :]  # [64, 512] f16
                for k in range(K):
                    u16 = sp.tile([128, 4, C], f16, tag="u16")
                    for m in range(4):
                        pt = pu.tile([128, C], f32, tag="pu")
                        nc.tensor.matmul(out=pt[:, :],
                                         lhsT=vt[:, m * 128:(m + 1) * 128],
                                         rhs=cw[off:off + C, k * C:(k + 1) * C],
                                         start=True, stop=True)
                        nc.scalar.copy(out=u16[:, m, :], in_=pt[:, :])
                    for m in range(4):
                        nc.sync.dma_start(out=UO[b, k, m], in_=u16[:, m, :])

            # ---- qn = q / ||q||_col, emitted as hi/lo f16 ----
            for b in range(B):
                off = (b % 2) * C
                q_b = qacc[b // 2][off:off + C, :]  # [64, 512] f32 psum
                qf = sp.tile([128, OS], f32, tag="qf")
                nc.scalar.copy(out=qf[off:off + C, :], in_=q_b)
                sq = sp.tile([128, OS], f32, tag="sq")
                nc.scalar.square(out=sq[off:off + C, :], in_=qf[off:off + C, :])
                pscr = pu.tile([128, OS], f32, tag="pscr", name="pscr")
                nc.tensor.matmul(out=pscr[0:1, :], lhsT=ones_r[off:off + C, :],
                                 rhs=sq[off:off + C, :], start=True, stop=True)
                nrm = sp.tile([1, OS], f32, tag="nrm")
                nc.scalar.sqrt(out=nrm[:, :], in_=pscr[0:1, :])
                rcp = sp.tile([1, OS], f32, tag="rcp")
                nc.vector.reciprocal(out=rcp[:, :], in_=nrm[:, :])
                pb = pu.tile([128, OS], f32, tag="pscr", name="pb")
                nc.tensor.matmul(out=pb[off:off + C, :], lhsT=ones_b[:, :],
                                 rhs=rcp[:, :], start=True, stop=True)
                bc = sp.tile([128, OS], f32, tag="bc")
                nc.scalar.copy(out=bc[off:off + C, :], in_=pb[off:off + C, :])
                qn = sp.tile([128, OS], f32, tag="qn")
                nc.vector.tensor_mul(out=qn[off:off + C, :], in0=qf[off:off + C, :],
                                     in1=bc[off:off + C, :])
                qh16 = sp.tile([128, OS], f16, tag="qh16")
                nc.scalar.copy(out=qh16[off:off + C, :], in_=qn[off:off + C, :])
                qh32 = sp.tile([128, OS], f32, tag="qh32")
                nc.scalar.copy(out=qh32[off:off + C, :], in_=qh16[off:off + C, :])
                ql16 = sp.tile([128, OS], f16, tag="ql16")
                nc.vector.tensor_sub(out=ql16[off:off + C, :],
                                     in0=qn[off:off + C, :], in1=qh32[off:off + C, :])
                nc.sync.dma_start(out=QNH[b * C:(b + 1) * C, :], in_=qh16[off:off + C, :])
                nc.sync.dma_start(out=QNL[b * C:(b + 1) * C, :], in_=ql16[off:off + C, :])
    nc.compile()
    return nc


def _build_l2():
    """Phase 2, SPMD core j: rows t in [j*512,(j+1)*512) for all batches.
    Exact sim via fp16 split-3 matmuls in 4 psum chunks of 1024; per-chunk
    exact top-8 via max/max_index on the f32 copy; merge 32 candidates via
    max8 + find_index + one-hot decode; gather-sum u tables -> yT;
    partial out = yT^T @ WoT slice."""
    nc = bacc.Bacc("TRN2", target_bir_lowering=False, debug=False, num_devices=NCORES)
    QNH = nc.dram_tensor("qnh", [BC, T], f16, kind="ExternalInput").ap()
    QNL = nc.dram_tensor("qnl", [BC, T], f16, kind="ExternalInput").ap()
    KHJ = nc.dram_tensor("khj", [BC, OS], f16, kind="ExternalInput").ap()
    KLJ = nc.dram_tensor("klj", [BC, OS], f16, kind="ExternalInput").ap()
    WOT = nc.dram_tensor("wot", [OS, T], f16, kind="ExternalInput").ap()
    IOTA = nc.dram_tensor("iota32", [128, 32], u32, kind="ExternalInput").ap()
    UT = [[nc.dram_tensor(f"ut{b}_{k}", [T, C], f16, kind="ExternalInput").ap()
           for k in range(K)] for b in range(B)]
    OUT = nc.dram_tensor("out_o", [2, 128, T], f32, kind="ExternalOutput").ap()

    NCH = 4          # sim chunks per block
    CW_ = T // NCH   # 1024 columns per chunk

    with tile.TileContext(nc) as tc:
        with tc.tile_pool(name="qp", bufs=1) as qp, \
             tc.tile_pool(name="wp", bufs=1) as wp, \
             tc.tile_pool(name="sp", bufs=3) as sp, \
             tc.tile_pool(name="simp", bufs=6) as simp, \
             tc.tile_pool(name="yp", bufs=1) as yp, \
             tc.tile_pool(name="obp", bufs=2) as obp, \
             tc.tile_pool(name="pp", bufs=4, space="PSUM") as pp, \
             tc.tile_pool(name="po", bufs=2, space="PSUM") as po:
            kh, kl = {}, {}
            for mt in range(2):
                h16 = qp.tile([128, OS], f16, tag=f"kh{mt}")
                nc.sync.dma_start(out=h16[:, :], in_=KHJ[mt * 128:(mt + 1) * 128, :])
                l16 = qp.tile([128, OS], f16, tag=f"kl{mt}")
                nc.sync.dma_start(out=l16[:, :], in_=KLJ[mt * 128:(mt + 1) * 128, :])
                kh[mt], kl[mt] = h16, l16
            qh, ql = {}, {}
            for mt in range(2):
                h16 = qp.tile([128, T], f16, tag=f"qh{mt}")
                l16 = qp.tile([128, T], f16, tag=f"ql{mt}")
                for cc in range(4):
                    cs = slice(cc * 1024, (cc + 1) * 1024)
                    nc.sync.dma_start(out=h16[:, cs], in_=QNH[mt * 128:(mt + 1) * 128, cs])
                    nc.sync.dma_start(out=l16[:, cs], in_=QNL[mt * 128:(mt + 1) * 128, cs])
                qh[mt], ql[mt] = h16, l16
            wot = {}
            iota = qp.tile([128, 32], u32, tag="iota")
            nc.sync.dma_start(out=iota[:, :], in_=IOTA[:, :])

            wu = qp.tile([128, 512], f16, tag="wu")
            nc.vector.memset(wu[:, :], 0.0)

            ytp = {}  # (pair, kt) -> [128, 128] f16 lhsT tiles for the out matmul
            for pair in range(2):
                for kt in range(4):
                    yt_tile = yp.tile([128, 128], f16, tag=f"yt{pair}{kt}")
                    ytp[(pair, kt)] = yt_tile

            for b in range(B):
                khb = kh[b // 2][(b % 2) * C:(b % 2) * C + C, :]   # [64, 512] f16
                klb = kl[b // 2][(b % 2) * C:(b % 2) * C + C, :]
                qhb = qh[b // 2][(b % 2) * C:(b % 2) * C + C, :]   # [64, 4096] f16
                qlb = ql[b // 2][(b % 2) * C:(b % 2) * C + C, :]
                for i in range(4):  # 128-token blocks of this core's slice
                    lh = khb[:, i * 128:(i + 1) * 128]
                    ll = klb[:, i * 128:(i + 1) * 128]
                    m32 = sp.tile([128, 32], f32, tag="m32")
                    simf = simp.tile([128, T], f32, tag="simf")
                    for ch in range(NCH):
                        for half in range(2):
                            c0 = ch * CW_ + half * 512
                            rh = qhb[:, c0:c0 + 512]
                            rl = qlb[:, c0:c0 + 512]
                            ps = pp.tile([128, 512], f32, tag="ps")
                            nc.tensor.matmul(out=ps[:, :], lhsT=lh, rhs=rh,
                                             start=True, stop=False)
                            nc.tensor.matmul(out=ps[:, :], lhsT=ll, rhs=rh,
                                             start=False, stop=False)
                            nc.tensor.matmul(out=ps[:, :], lhsT=lh, rhs=rl,
                                             start=False, stop=True)
                            nc.scalar.copy(out=simf[:, c0:c0 + 512], in_=ps[:, :])
                        nc.vector.max(out=m32[:, ch * 8:(ch + 1) * 8],
                                      in_=simf[:, ch * CW_:(ch + 1) * CW_])
                    # merge: global top-8 values, then one full-row index scan
                    g8 = sp.tile([128, 8], f32, tag="g8")
                    nc.vector.max(out=g8[:, :], in_=m32[:, :])
                    idx8 = sp.tile([128, 8], u32, tag="idx8")
                    nc.vector.max_index(out=idx8[:, :], in_max=g8[:, :],
                                        in_values=simf[:, :])
                    gth = sp.tile([128, K, C], f16, tag="gth")
                    for k in range(K):
                        nc.gpsimd.indirect_dma_start(
                            out=gth[:, k, :], out_offset=None,
                            in_=UT[b][k][:, :],
                            in_offset=bass.IndirectOffsetOnAxis(ap=idx8[:, k:k + 1], axis=0))
                    t0 = sp.tile([128, C], f16, tag="t0")
                    t1 = sp.tile([128, C], f16, tag="t1")
                    nc.gpsimd.tensor_add(out=t0[:, :], in0=gth[:, 0, :], in1=gth[:, 1, :])
                    nc.gpsimd.tensor_add(out=t1[:, :], in0=gth[:, 2, :], in1=gth[:, 3, :])
                    dst = ytp[(b // 2, i)][:, (b % 2) * C:(b % 2) * C + C]
                    nc.gpsimd.tensor_add(out=dst, in0=t0[:, :], in1=t1[:, :])

                # emit the pair's out-projection as soon as both batches done
                if b % 2 == 1:
                    pair = b // 2
                    if pair == 1:
                        # keep PE warm through the tail (no-dep matmuls)
                        for _w in range(10):
                            pw = pp.tile([128, 512], f32, tag="ps")
                            nc.tensor.matmul(out=pw[:, :], lhsT=wu[:, :128],
                                             rhs=wu[:, :], start=True, stop=True)
                    if not wot:
                        for kt in range(4):
                            w = wp.tile([128, T], f16, tag=f"wot{kt}", name=f"wot{kt}")
                            nc.sync.dma_start(out=w[:, :], in_=WOT[kt * 128:(kt + 1) * 128, :])
                            wot[kt] = w
                    ob = obp.tile([128, T], f32, tag="ob")
                    for ch in range(NCH):
                        ps = po.tile([128, CW_], f32, tag="po")
                        for half in range(2):
                            c0 = ch * CW_ + half * 512
                            po_s = ps[:, half * 512:(half + 1) * 512]
                            for kt in range(4):
                                nc.tensor.matmul(out=po_s, lhsT=ytp[(pair, kt)][:, :],
                                                 rhs=wot[kt][:, c0:c0 + 512],
                                                 start=(kt == 0), stop=(kt == 3))
                        nc.scalar.copy(out=ob[:, ch * CW_:(ch + 1) * CW_], in_=ps[:, :])
                        nc.sync.dma_start(out=OUT[pair, :, ch * CW_:(ch + 1) * CW_],
                                          in_=ob[:, ch * CW_:(ch + 1) * CW_])


    nc.compile()
    return nc


def _split16(a):
    h = a.astype(np.float16)
    l = (a - h.astype(np.float32)).astype(np.float16)
    return h, l


def kernel(x, Wq, Wk, Wv, Wo, conv_w, conv_b):
    x = np.asarray(x, np.float32)
    Wq = np.asarray(Wq, np.float32); Wk = np.asarray(Wk, np.float32)
    Wv = np.asarray(Wv, np.float32); Wo = np.asarray(Wo, np.float32)
    conv_w = np.asarray(conv_w, np.float32); conv_b = np.asarray(conv_b, np.float32)

    LAST_EXEC_NS.clear(); LAST_TRACE_DIRS.clear()
    if "l1" not in _cache:
        _cache["l1"] = _build_l1()
    if "l2" not in _cache:
        _cache["l2"] = _build_l2()

    def _sw(a):
        # [T, W] -> [128, T//128, W] with [p, kt, w] = a[kt*128+p, w]
        return np.ascontiguousarray(a.reshape(T // 128, 128, -1).transpose(1, 0, 2))

    xT = np.ascontiguousarray(x.transpose(2, 0, 1).reshape(T, BC))  # [t, b*64+c]
    xh, xl = _split16(xT)
    xh, xl = _sw(xh), _sw(xl)
    WqT, WkT = Wq.T, Wk.T
    WvT16 = np.ascontiguousarray(Wv.T).astype(np.float16)
    cw1 = np.ascontiguousarray(conv_w.transpose(1, 2, 0).reshape(C, K * C)).astype(np.float16)
    cw = np.concatenate([cw1, cw1], axis=0)
    # cw[ci, k*64+co] = conv_w[co, ci, k]

    in_maps = []
    for j in range(NCORES):
        sl = slice(j * OS, (j + 1) * OS)
        wqh, wql = _split16(np.ascontiguousarray(WqT[:, sl]))
        wkh, wkl = _split16(np.ascontiguousarray(WkT[:, sl]))
        in_maps.append({"xh": xh, "xl": xl,
                        "wqh": _sw(wqh), "wql": _sw(wql),
                        "wkh": _sw(wkh), "wkl": _sw(wkl),
                        "wv": _sw(np.ascontiguousarray(WvT16[:, sl])), "cw": cw})
    r1 = _run(_cache["l1"], in_maps)

    qnh = np.concatenate([r1.results[j]["qnh_o"] for j in range(NCORES)], axis=1)
    qnl = np.concatenate([r1.results[j]["qnl_o"] for j in range(NCORES)], axis=1)
    ut = {}
    for b in range(B):
        for k in range(K):
            ut[(b, k)] = np.ascontiguousarray(np.concatenate(
                [r1.results[j]["u_o"][b, k].reshape(OS, C) for j in range(NCORES)], axis=0))

    iota32 = np.broadcast_to(np.arange(32, dtype=np.uint32), (128, 32)).copy()
    in_maps2 = []
    for j in range(NCORES):
        m = {"qnh": qnh, "qnl": qnl,
             "khj": r1.results[j]["kh_o"], "klj": r1.results[j]["kl_o"],
             "wot": np.ascontiguousarray(Wo.T[j * OS:(j + 1) * OS, :]).astype(np.float16),
             "iota32": iota32}
        for b in range(B):
            for k in range(K):
                m[f"ut{b}_{k}"] = ut[(b, k)]
        in_maps2.append(m)
    r2 = _run(_cache["l2"], in_maps2)

    out = np.zeros((B, C, T), np.float32)
    for j in range(NCORES):
        oo = r2.results[j]["out_o"]  # [2, 128, T]
        for b in range(B):
            out[b] += oo[b // 2, (b % 2) * C:(b % 2) * C + C, :]
    bias = conv_b[:, None] * Wo.sum(axis=1)[None, :]  # [64, 4096]
    out += bias[None, :, :]
    return out


# revision 35
# speedup vs baseline: 1.1019x; 1.0299x over previous
import sys
for p in ('/opt/trn_rl_repo', '/opt/pypackages'):
    if p not in sys.path:
        sys.path.insert(0, p)
import numpy as np
from concourse import bass, bacc, tile, mybir
from concourse import bass_utils

B, C, T, K = 4, 64, 4096, 4
NCORES = 8
OS = T // NCORES          # 512: per-core token slice (phase-1 o-slice == phase-2 t-slice)
BC = B * C                # 256
f32 = mybir.dt.float32
f16 = mybir.dt.float16
u32 = mybir.dt.uint32

_cache = {}
LAST_EXEC_NS = []
LAST_TRACE_DIRS = []


def _run(nc, in_maps):
    r = bass_utils.run_bass_kernel_spmd(nc, in_maps, core_ids=list(range(NCORES)))
    LAST_EXEC_NS.append(getattr(r, "exec_time_ns", None))
    it = getattr(r, "instructions_and_trace", None)
    if it is not None:
        LAST_TRACE_DIRS.append(it[1])
    return r


def _build_l1():
    """Phase 1, SPMD core j: single kt-sweep computing q,k (fp16 split-3) and v
    (fp16) o-slices with grouped stationary operands; emits qn hi/lo f16,
    k hi/lo f16 and the conv-folded value tables u."""
    nc = bacc.Bacc("TRN2", target_bir_lowering=False, debug=False, num_devices=NCORES)
    XH = nc.dram_tensor("xh", [128, T // 128, BC], f16, kind="ExternalInput").ap()
    XL = nc.dram_tensor("xl", [128, T // 128, BC], f16, kind="ExternalInput").ap()
    WQH = nc.dram_tensor("wqh", [128, T // 128, OS], f16, kind="ExternalInput").ap()
    WQL = nc.dram_tensor("wql", [128, T // 128, OS], f16, kind="ExternalInput").ap()
    WKH = nc.dram_tensor("wkh", [128, T // 128, OS], f16, kind="ExternalInput").ap()
    WKL = nc.dram_tensor("wkl", [128, T // 128, OS], f16, kind="ExternalInput").ap()
    WV = nc.dram_tensor("wv", [128, T // 128, OS], f16, kind="ExternalInput").ap()
    CW = nc.dram_tensor("cw", [2 * C, K * C], f16, kind="ExternalInput").ap()
    QNH = nc.dram_tensor("qnh_o", [BC, OS], f16, kind="ExternalOutput").ap()
    QNL = nc.dram_tensor("qnl_o", [BC, OS], f16, kind="ExternalOutput").ap()
    KH = nc.dram_tensor("kh_o", [BC, OS], f16, kind="ExternalOutput").ap()
    KL = nc.dram_tensor("kl_o", [BC, OS], f16, kind="ExternalOutput").ap()
    UO = nc.dram_tensor("u_o", [B, K, 4, 128, C], f16, kind="ExternalOutput").ap()

    NKT = T // 128  # 32 contraction tiles
    NHF = 16
    H = NKT // NHF  # 8 kt per weight chunk

    with tile.TileContext(nc) as tc:
        with tc.tile_pool(name="xp", bufs=1) as xp, \
             tc.tile_pool(name="wp", bufs=4) as wp, \
             tc.tile_pool(name="sp", bufs=2) as sp, \
             tc.tile_pool(name="cp", bufs=1) as cp, \
             tc.tile_pool(name="pp", bufs=1, space="PSUM") as pp, \
             tc.tile_pool(name="pu", bufs=1, space="PSUM") as pu:
            xh = xp.tile([128, NKT, BC], f16, tag="xh")
            xl = xp.tile([128, NKT, BC], f16, tag="xl")
            cw = cp.tile([2 * C, K * C], f16, tag="cw")
            nc.sync.dma_start(out=cw[:, :], in_=CW[:, :])
            ones_r = cp.tile([128, 1], f32, tag="ones_r")   # reduce lhsT
            nc.vector.memset(ones_r[:, :], 1.0)
            ones_b = cp.tile([1, C], f32, tag="ones_b")   # broadcast lhsT
            nc.vector.memset(ones_b[:, :], 1.0)

            wu1 = cp.tile([128, 512], f16, tag="wu1")
            nc.vector.memset(wu1[:, :], 0.0)
            for _w in range(6):
                pw = pu.tile([128, OS], f32, tag="pscr", name=f"warm{_w}")
                nc.tensor.matmul(out=pw[:, :], lhsT=wu1[:, :128], rhs=wu1[:, :],
                                 start=True, stop=True)

            qacc, kacc, vacc = [], [], []
            for mt in range(2):
                qacc.append(pp.tile([128, OS], f32, tag=f"qa{mt}", name=f"qa{mt}"))
                kacc.append(pp.tile([128, OS], f32, tag=f"ka{mt}", name=f"ka{mt}"))
                vacc.append(pp.tile([128, OS], f32, tag=f"va{mt}", name=f"va{mt}"))

            for hf in range(NHF):
                slx = slice(hf * H, (hf + 1) * H)
                nc.sync.dma_start(out=xh[:, slx, :], in_=XH[:, slx, :])
                nc.sync.dma_start(out=xl[:, slx, :], in_=XL[:, slx, :])
                wqh = wp.tile([128, H, OS], f16, tag="wqh")
                wql = wp.tile([128, H, OS], f16, tag="wql")
                wkh = wp.tile([128, H, OS], f16, tag="wkh")
                wkl = wp.tile([128, H, OS], f16, tag="wkl")
                wv = wp.tile([128, H, OS], f16, tag="wv")
                sl = slice(hf * H, (hf + 1) * H)
                nc.sync.dma_start(out=wqh[:, :, :], in_=WQH[:, sl, :])
                nc.sync.dma_start(out=wql[:, :, :], in_=WQL[:, sl, :])
                nc.sync.dma_start(out=wkh[:, :, :], in_=WKH[:, sl, :])
                nc.sync.dma_start(out=wkl[:, :, :], in_=WKL[:, sl, :])
                nc.sync.dma_start(out=wv[:, :, :], in_=WV[:, sl, :])
                for t in range(H):
                    kt = hf * H + t
                    first = (kt == 0)
                    last = (kt == NKT - 1)
                    for mt in range(2):
                        lh = xh[:, kt, mt * 128:(mt + 1) * 128]
                        ll = xl[:, kt, mt * 128:(mt + 1) * 128]
                        # lh-stationary group
                        nc.tensor.matmul(out=qacc[mt][:, :], lhsT=lh, rhs=wqh[:, t, :],
                                         start=first, stop=False)
                        nc.tensor.matmul(out=qacc[mt][:, :], lhsT=lh, rhs=wql[:, t, :],
                                         start=False, stop=False)
                        nc.tensor.matmul(out=kacc[mt][:, :], lhsT=lh, rhs=wkh[:, t, :],
                                         start=first, stop=False)
                        nc.tensor.matmul(out=kacc[mt][:, :], lhsT=lh, rhs=wkl[:, t, :],
                                         start=False, stop=False)
                        nc.tensor.matmul(out=vacc[mt][:, :], lhsT=lh, rhs=wv[:, t, :],
                                         start=first, stop=last)
                        # ll-stationary group
                        nc.tensor.matmul(out=qacc[mt][:, :], lhsT=ll, rhs=wqh[:, t, :],
                                         start=False, stop=last)
                        nc.tensor.matmul(out=kacc[mt][:, :], lhsT=ll, rhs=wkh[:, t, :],
                                         start=False, stop=last)

            # ---- k: split hi/lo f16 and store ----
            for mt in range(2):
                kf = sp.tile([128, OS], f32, tag=f"kf{mt}")
                nc.scalar.copy(out=kf[:, :], in_=kacc[mt][:, :])
                kh16 = sp.tile([128, OS], f16, tag=f"kh{mt}")
                nc.scalar.copy(out=kh16[:, :], in_=kf[:, :])
                kh32 = sp.tile([128, OS], f32, tag=f"kh32{mt}")
                nc.scalar.copy(out=kh32[:, :], in_=kh16[:, :])
                kl16 = sp.tile([128, OS], f16, tag=f"kl{mt}")
                nc.vector.tensor_sub(out=kl16[:, :], in0=kf[:, :], in1=kh32[:, :])
                nc.sync.dma_start(out=KH[mt * 128:(mt + 1) * 128, :], in_=kh16[:, :])
                nc.sync.dma_start(out=KL[mt * 128:(mt + 1) * 128, :], in_=kl16[:, :])

            # ---- v to f16; u tables: u[b,k] = (v_b^T @ cw_k) as 4 M-tiles ----
            vsb = {}
            for mt in range(2):
                v16 = sp.tile([128, OS], f16, tag=f"vsb{mt}")
                nc.scalar.copy(out=v16[:, :], in_=vacc[mt][:, :])
                vsb[mt] = v16
            for b in range(B):
                off = (b % 2) * C
                vt = vsb[b // 2][off:off + C, :]  # [64, 512] f16
                for k in range(K):
                    u16 = sp.tile([128, 4, C], f16, tag="u16")
                    for m in range(4):
                        pt = pu.tile([128, C], f32, tag="pu")
                        nc.tensor.matmul(out=pt[:, :],
                                         lhsT=vt[:, m * 128:(m + 1) * 128],
                                         rhs=cw[off:off + C, k * C:(k + 1) * C],
                                         start=True, stop=True)
                        nc.scalar.copy(out=u16[:, m, :], in_=pt[:, :])
                    for m in range(4):
                        nc.sync.dma_start(out=UO[b, k, m], in_=u16[:, m, :])

            # ---- qn = q / ||q||_col, emitted as hi/lo f16 ----
            for b in range(B):
                off = (b % 2) * C
                q_b = qacc[b // 2][off:off + C, :]  # [64, 512] f32 psum
                qf = sp.tile([128, OS], f32, tag="qf")
                nc.scalar.copy(out=qf[off:off + C, :], in_=q_b)
                sq = sp.tile([128, OS], f32, tag="sq")
                nc.scalar.square(out=sq[off:off + C, :], in_=qf[off:off + C, :])
                pscr = pu.tile([128, OS], f32, tag="pscr", name="pscr")
                nc.tensor.matmul(out=pscr[0:1, :], lhsT=ones_r[off:off + C, :],
                                 rhs=sq[off:off + C, :], start=True, stop=True)
                nrm = sp.tile([1, OS], f32, tag="nrm")
                nc.scalar.sqrt(out=nrm[:, :], in_=pscr[0:1, :])
                rcp = sp.tile([1, OS], f32, tag="rcp")
                nc.vector.reciprocal(out=rcp[:, :], in_=nrm[:, :])
                pb = pu.tile([128, OS], f32, tag="pscr", name="pb")
                nc.tensor.matmul(out=pb[off:off + C, :], lhsT=ones_b[:, :],
                                 rhs=rcp[:, :], start=True, stop=True)
                bc = sp.tile([128, OS], f32, tag="bc")
                nc.scalar.copy(out=bc[off:off + C, :], in_=pb[off:off + C, :])
                qn = sp.tile([128, OS], f32, tag="qn")
                nc.vector.tensor_mul(out=qn[off:off + C, :], in0=qf[off:off + C, :],
                                     in1=bc[off:off + C, :])
                qh16 = sp.tile([128, OS], f16, tag="qh16")
                nc.scalar.copy(out=qh16[off:off + C, :], in_=qn[off:off + C, :])
                qh32 = sp.tile([128, OS], f32, tag="qh32")
                nc.scalar.copy(out=qh32[off:off + C, :], in_=qh16[off:off + C, :])
                ql16 = sp.tile([128, OS], f16, tag="ql16")
                nc.vector.tensor_sub(out=ql16[off:off + C, :],
                                     in0=qn[off:off + C, :], in1=qh32[off:off + C, :])
                nc.sync.dma_start(out=QNH[b * C:(b + 1) * C, :], in_=qh16[off:off + C, :])
                nc.sync.dma_start(out=QNL[b * C:(b + 1) * C, :], in_=ql16[off:off + C, :])
    nc.compile()
    return nc


def _build_l2():
    """Phase 2, SPMD core j: rows t in [j*512,(j+1)*512) for all batches.
    Exact sim via fp16 split-3 matmuls in 4 psum chunks of 1024; per-chunk
    exact top-8 via max/max_index on the f32 copy; merge 32 candidates via
    max8 + find_index + one-hot decode; gather-sum u tables -> yT;
    partial out = yT^T @ WoT slice."""
    nc = bacc.Bacc("TRN2", target_bir_lowering=False, debug=False, num_devices=NCORES)
    QNH = nc.dram_tensor("qnh", [BC, T], f16, kind="ExternalInput").ap()
    QNL = nc.dram_tensor("qnl", [BC, T], f16, kind="ExternalInput").ap()
    KHJ = nc.dram_tensor("khj", [BC, OS], f16, kind="ExternalInput").ap()
    KLJ = nc.dram_tensor("klj", [BC, OS], f16, kind="ExternalInput").ap()
    WOT = nc.dram_tensor("wot", [OS, T], f16, kind="ExternalInput").ap()
    IOTA = nc.dram_tensor("iota32", [128, 32], u32, kind="ExternalInput").ap()
    UT = [[nc.dram_tensor(f"ut{b}_{k}", [T, C], f16, kind="ExternalInput").ap()
           for k in range(K)] for b in range(B)]
    OUT = nc.dram_tensor("out_o", [2, 128, T], f32, kind="ExternalOutput").ap()

    NCH = 4          # sim chunks per block
    CW_ = T // NCH   # 1024 columns per chunk

    with tile.TileContext(nc) as tc:
        with tc.tile_pool(name="qp", bufs=1) as qp, \
             tc.tile_pool(name="wp", bufs=1) as wp, \
             tc.tile_pool(name="sp", bufs=3) as sp, \
             tc.tile_pool(name="simp", bufs=6) as simp, \
             tc.tile_pool(name="yp", bufs=1) as yp, \
             tc.tile_pool(name="obp", bufs=2) as obp, \
             tc.tile_pool(name="pp", bufs=4, space="PSUM") as pp, \
             tc.tile_pool(name="po", bufs=2, space="PSUM") as po:
            kh, kl = {}, {}
            for mt in range(2):
                h16 = qp.tile([128, OS], f16, tag=f"kh{mt}")
                nc.sync.dma_start(out=h16[:, :], in_=KHJ[mt * 128:(mt + 1) * 128, :])
                l16 = qp.tile([128, OS], f16, tag=f"kl{mt}")
                nc.sync.dma_start(out=l16[:, :], in_=KLJ[mt * 128:(mt + 1) * 128, :])
                kh[mt], kl[mt] = h16, l16
            qh, ql = {}, {}
            for mt in range(2):
                h16 = qp.tile([128, T], f16, tag=f"qh{mt}")
                l16 = qp.tile([128, T], f16, tag=f"ql{mt}")
                for cc in range(4):
                    cs = slice(cc * 1024, (cc + 1) * 1024)
                    nc.sync.dma_start(out=h16[:, cs], in_=QNH[mt * 128:(mt + 1) * 128, cs])
                    nc.sync.dma_start(out=l16[:, cs], in_=QNL[mt * 128:(mt + 1) * 128, cs])
                qh[mt], ql[mt] = h16, l16
            wot = {}
            iota = qp.tile([128, 32], u32, tag="iota")
            nc.sync.dma_start(out=iota[:, :], in_=IOTA[:, :])

            wu = qp.tile([128, 512], f16, tag="wu")
            nc.vector.memset(wu[:, :], 0.0)

            ytp = {}  # (pair, kt) -> [128, 128] f16 lhsT tiles for the out matmul
            for pair in range(2):
                for kt in range(4):
                    yt_tile = yp.tile([128, 128], f16, tag=f"yt{pair}{kt}")
                    ytp[(pair, kt)] = yt_tile

            for b in range(B):
                khb = kh[b // 2][(b % 2) * C:(b % 2) * C + C, :]   # [64, 512] f16
                klb = kl[b // 2][(b % 2) * C:(b % 2) * C + C, :]
                qhb = qh[b // 2][(b % 2) * C:(b % 2) * C + C, :]   # [64, 4096] f16
                qlb = ql[b // 2][(b % 2) * C:(b % 2) * C + C, :]
                for i in range(4):  # 128-token blocks of this core's slice
                    lh = khb[:, i * 128:(i + 1) * 128]
                    ll = klb[:, i * 128:(i + 1) * 128]
                    m32 = sp.tile([128, 32], f32, tag="m32")
                    simf = simp.tile([128, T], f32, tag="simf")
                    for ch in range(NCH):
                        for half in range(2):
                            c0 = ch * CW_ + half * 512
                            rh = qhb[:, c0:c0 + 512]
                            rl = qlb[:, c0:c0 + 512]
                            ps = pp.tile([128, 512], f32, tag="ps")
                            nc.tensor.matmul(out=ps[:, :], lhsT=lh, rhs=rh,
                                             start=True, stop=False)
                            nc.tensor.matmul(out=ps[:, :], lhsT=ll, rhs=rh,
                                             start=False, stop=False)
                            nc.tensor.matmul(out=ps[:, :], lhsT=lh, rhs=rl,
                                             start=False, stop=True)
                            nc.scalar.copy(out=simf[:, c0:c0 + 512], in_=ps[:, :])
                        nc.vector.max(out=m32[:, ch * 8:(ch + 1) * 8],
                                      in_=simf[:, ch * CW_:(ch + 1) * CW_])
                    # merge: global top-8 values, then one full-row index scan
                    g8 = sp.tile([128, 8], f32, tag="g8")
                    nc.vector.max(out=g8[:, :], in_=m32[:, :])
                    idx8 = sp.tile([128, 8], u32, tag="idx8")
                    nc.vector.max_index(out=idx8[:, :], in_max=g8[:, :],
                                        in_values=simf[:, :])
                    gth = sp.tile([128, K, C], f16, tag="gth")
                    for k in range(K):
                        nc.gpsimd.indirect_dma_start(
                            out=gth[:, k, :], out_offset=None,
                            in_=UT[b][k][:, :],
                            in_offset=bass.IndirectOffsetOnAxis(ap=idx8[:, k:k + 1], axis=0))
                    t0 = sp.tile([128, C], f16, tag="t0")
                    t1 = sp.tile([128, C], f16, tag="t1")
                    nc.gpsimd.tensor_add(out=t0[:, :], in0=gth[:, 0, :], in1=gth[:, 1, :])
                    nc.gpsimd.tensor_add(out=t1[:, :], in0=gth[:, 2, :], in1=gth[:, 3, :])
                    dst = ytp[(b // 2, i)][:, (b % 2) * C:(b % 2) * C + C]
                    nc.gpsimd.tensor_add(out=dst, in0=t0[:, :], in1=t1[:, :])

                # emit the pair's out-projection as soon as both batches done
                if b % 2 == 1:
                    pair = b // 2
                    if pair == 1:
                        # keep PE warm through the tail (no-dep matmuls)
                        for _w in range(10):
                            pw = pp.tile([128, 512], f32, tag="ps")
                            nc.tensor.matmul(out=pw[:, :], lhsT=wu[:, :128],
                                             rhs=wu[:, :], start=True, stop=True)
                    if not wot:
                        for kt in range(4):
                            w = wp.tile([128, T], f16, tag=f"wot{kt}", name=f"wot{kt}")
                            nc.sync.dma_start(out=w[:, :], in_=WOT[kt * 128:(kt + 1) * 128, :])
                            wot[kt] = w
                    ob = obp.tile([128, T], f32, tag="ob")
                    for ch in range(NCH):
                        ps = po.tile([128, CW_], f32, tag="po")
                        for half in range(2):
                            c0 = ch * CW_ + half * 512
                            po_s = ps[:, half * 512:(half + 1) * 512]
                            for kt in range(4):
                                nc.tensor.matmul(out=po_s, lhsT=ytp[(pair, kt)][:, :],
                                                 rhs=wot[kt][:, c0:c0 + 512],
                                                 start=(kt == 0), stop=(kt == 3))
                        nc.scalar.copy(out=ob[:, ch * CW_:(ch + 1) * CW_], in_=ps[:, :])
                        nc.sync.dma_start(out=OUT[pair, :, ch * CW_:(ch + 1) * CW_],
                                          in_=ob[:, ch * CW_:(ch + 1) * CW_])


    nc.compile()
    return nc


def _split16(a):
    h = a.astype(np.float16)
    l = (a - h.astype(np.float32)).astype(np.float16)
    return h, l


def kernel(x, Wq, Wk, Wv, Wo, conv_w, conv_b):
    x = np.asarray(x, np.float32)
    Wq = np.asarray(Wq, np.float32); Wk = np.asarray(Wk, np.float32)
    Wv = np.asarray(Wv, np.float32); Wo = np.asarray(Wo, np.float32)
    conv_w = np.asarray(conv_w, np.float32); conv_b = np.asarray(conv_b, np.float32)

    LAST_EXEC_NS.clear(); LAST_TRACE_DIRS.clear()
    if "l1" not in _cache:
        _cache["l1"] = _build_l1()
    if "l2" not in _cache:
        _cache["l2"] = _build_l2()

    def _sw(a):
        # [T, W] -> [128, T//128, W] with [p, kt, w] = a[kt*128+p, w]
        return np.ascontiguousarray(a.reshape(T // 128, 128, -1).transpose(1, 0, 2))

    xT = np.ascontiguousarray(x.transpose(2, 0, 1).reshape(T, BC))  # [t, b*64+c]
    xh, xl = _split16(xT)
    xh, xl = _sw(xh), _sw(xl)
    WqT, WkT = Wq.T, Wk.T
    WvT16 = np.ascontiguousarray(Wv.T).astype(np.float16)
    cw1 = np.ascontiguousarray(conv_w.transpose(1, 2, 0).reshape(C, K * C)).astype(np.float16)
    cw = np.concatenate([cw1, cw1], axis=0)
    # cw[ci, k*64+co] = conv_w[co, ci, k]

    in_maps = []
    for j in range(NCORES):
        sl = slice(j * OS, (j + 1) * OS)
        wqh, wql = _split16(np.ascontiguousarray(WqT[:, sl]))
        wkh, wkl = _split16(np.ascontiguousarray(WkT[:, sl]))
        in_maps.append({"xh": xh, "xl": xl,
                        "wqh": _sw(wqh), "wql": _sw(wql),
                        "wkh": _sw(wkh), "wkl": _sw(wkl),
                        "wv": _sw(np.ascontiguousarray(WvT16[:, sl])), "cw": cw})
    r1 = _run(_cache["l1"], in_maps)

    qnh = np.concatenate([r1.results[j]["qnh_o"] for j in range(NCORES)], axis=1)
    qnl = np.concatenate([r1.results[j]["qnl_o"] for j in range(NCORES)], axis=1)
    ut = {}
    for b in range(B):
        for k in range(K):
            ut[(b, k)] = np.ascontiguousarray(np.concatenate(
                [r1.results[j]["u_o"][b, k].reshape(OS, C) for j in range(NCORES)], axis=0))

    iota32 = np.broadcast_to(np.arange(32, dtype=np.uint32), (128, 32)).copy()
    in_maps2 = []
    for j in range(NCORES):
        m = {"qnh": qnh, "qnl": qnl,
             "khj": r1.results[j]["kh_o"], "klj": r1.results[j]["kl_o"],
             "wot": np.ascontiguousarray(Wo.T[j * OS:(j + 1) * OS, :]).astype(np.float16),
             "iota32": iota32}
        for b in range(B):
            for k in range(K):
                m[f"ut{b}_{k}"] = ut[(b, k)]
        in_maps2.append(m)
    r2 = _run(_cache["l2"], in_maps2)

    out = np.zeros((B, C, T), np.float32)
    for j in range(NCORES):
        oo = r2.results[j]["out_o"]  # [2, 128, T]
        for b in range(B):
            out[b] += oo[b // 2, (b % 2) * C:(b % 2) * C + C, :]
    bias = conv_b[:, None] * Wo.sum(axis=1)[None, :]  # [64, 4096]
    out += bias[None, :, :]
    return out


# revision 38
# speedup vs baseline: 1.1056x; 1.0033x over previous
import sys
for p in ('/opt/trn_rl_repo', '/opt/pypackages'):
    if p not in sys.path:
        sys.path.insert(0, p)
import numpy as np
from concourse import bass, bacc, tile, mybir
from concourse import bass_utils

B, C, T, K = 4, 64, 4096, 4
NCORES = 8
OS = T // NCORES          # 512: per-core token slice (phase-1 o-slice == phase-2 t-slice)
BC = B * C                # 256
f32 = mybir.dt.float32
f16 = mybir.dt.float16
u32 = mybir.dt.uint32

_cache = {}
LAST_EXEC_NS = []
LAST_TRACE_DIRS = []


def _run(nc, in_maps):
    r = bass_utils.run_bass_kernel_spmd(nc, in_maps, core_ids=list(range(NCORES)))
    LAST_EXEC_NS.append(getattr(r, "exec_time_ns", None))
    it = getattr(r, "instructions_and_trace", None)
    if it is not None:
        LAST_TRACE_DIRS.append(it[1])
    return r


def _build_l1():
    """Phase 1, SPMD core j: single kt-sweep computing q,k (fp16 split-3) and v
    (fp16) o-slices with grouped stationary operands; emits qn hi/lo f16,
    k hi/lo f16 and the conv-folded value tables u."""
    nc = bacc.Bacc("TRN2", target_bir_lowering=False, debug=False, num_devices=NCORES)
    XH = nc.dram_tensor("xh", [128, T // 128, BC], f16, kind="ExternalInput").ap()
    XL = nc.dram_tensor("xl", [128, T // 128, BC], f16, kind="ExternalInput").ap()
    WQH = nc.dram_tensor("wqh", [128, T // 128, OS], f16, kind="ExternalInput").ap()
    WQL = nc.dram_tensor("wql", [128, T // 128, OS], f16, kind="ExternalInput").ap()
    WKH = nc.dram_tensor("wkh", [128, T // 128, OS], f16, kind="ExternalInput").ap()
    WKL = nc.dram_tensor("wkl", [128, T // 128, OS], f16, kind="ExternalInput").ap()
    WV = nc.dram_tensor("wv", [128, T // 128, OS], f16, kind="ExternalInput").ap()
    CW = nc.dram_tensor("cw", [2 * C, K * C], f16, kind="ExternalInput").ap()
    QNH = nc.dram_tensor("qnh_o", [BC, OS], f16, kind="ExternalOutput").ap()
    QNL = nc.dram_tensor("qnl_o", [BC, OS], f16, kind="ExternalOutput").ap()
    KH = nc.dram_tensor("kh_o", [BC, OS], f16, kind="ExternalOutput").ap()
    KL = nc.dram_tensor("kl_o", [BC, OS], f16, kind="ExternalOutput").ap()
    UO = nc.dram_tensor("u_o", [B, K, 4, 128, C], f16, kind="ExternalOutput").ap()

    NKT = T // 128  # 32 contraction tiles
    NHF = 16
    H = NKT // NHF  # 8 kt per weight chunk

    with tile.TileContext(nc) as tc:
        with tc.tile_pool(name="xp", bufs=1) as xp, \
             tc.tile_pool(name="wp", bufs=4) as wp, \
             tc.tile_pool(name="sp", bufs=2) as sp, \
             tc.tile_pool(name="cp", bufs=1) as cp, \
             tc.tile_pool(name="pp", bufs=1, space="PSUM") as pp, \
             tc.tile_pool(name="pu", bufs=1, space="PSUM") as pu:
            xh = xp.tile([128, NKT, BC], f16, tag="xh")
            xl = xp.tile([128, NKT, BC], f16, tag="xl")
            cw = cp.tile([2 * C, K * C], f16, tag="cw")
            nc.sync.dma_start(out=cw[:, :], in_=CW[:, :])
            ones_r = cp.tile([128, 1], f32, tag="ones_r")   # reduce lhsT
            nc.vector.memset(ones_r[:, :], 1.0)
            ones_b = cp.tile([1, C], f32, tag="ones_b")   # broadcast lhsT
            nc.vector.memset(ones_b[:, :], 1.0)

            wu1 = cp.tile([128, 512], f16, tag="wu1")
            nc.vector.memset(wu1[:, :], 0.0)
            for _w in range(6):
                pw = pu.tile([128, OS], f32, tag="pscr", name=f"warm{_w}")
                nc.tensor.matmul(out=pw[:, :], lhsT=wu1[:, :128], rhs=wu1[:, :],
                                 start=True, stop=True)

            qacc, kacc, vacc = [], [], []
            for mt in range(2):
                qacc.append(pp.tile([128, OS], f32, tag=f"qa{mt}", name=f"qa{mt}"))
                kacc.append(pp.tile([128, OS], f32, tag=f"ka{mt}", name=f"ka{mt}"))
                vacc.append(pp.tile([128, OS], f32, tag=f"va{mt}", name=f"va{mt}"))

            for hf in range(NHF):
                slx = slice(hf * H, (hf + 1) * H)
                nc.sync.dma_start(out=xh[:, slx, :], in_=XH[:, slx, :])
                nc.sync.dma_start(out=xl[:, slx, :], in_=XL[:, slx, :])
                wqh = wp.tile([128, H, OS], f16, tag="wqh")
                wql = wp.tile([128, H, OS], f16, tag="wql")
                wkh = wp.tile([128, H, OS], f16, tag="wkh")
                wkl = wp.tile([128, H, OS], f16, tag="wkl")
                wv = wp.tile([128, H, OS], f16, tag="wv")
                sl = slice(hf * H, (hf + 1) * H)
                nc.sync.dma_start(out=wqh[:, :, :], in_=WQH[:, sl, :])
                nc.sync.dma_start(out=wql[:, :, :], in_=WQL[:, sl, :])
                nc.sync.dma_start(out=wkh[:, :, :], in_=WKH[:, sl, :])
                nc.sync.dma_start(out=wkl[:, :, :], in_=WKL[:, sl, :])
                nc.sync.dma_start(out=wv[:, :, :], in_=WV[:, sl, :])
                for t in range(H):
                    kt = hf * H + t
                    first = (kt == 0)
                    last = (kt == NKT - 1)
                    for mt in range(2):
                        lh = xh[:, kt, mt * 128:(mt + 1) * 128]
                        ll = xl[:, kt, mt * 128:(mt + 1) * 128]
                        # lh-stationary group
                        nc.tensor.matmul(out=qacc[mt][:, :], lhsT=lh, rhs=wqh[:, t, :],
                                         start=first, stop=False)
                        nc.tensor.matmul(out=qacc[mt][:, :], lhsT=lh, rhs=wql[:, t, :],
                                         start=False, stop=False)
                        nc.tensor.matmul(out=kacc[mt][:, :], lhsT=lh, rhs=wkh[:, t, :],
                                         start=first, stop=False)
                        nc.tensor.matmul(out=kacc[mt][:, :], lhsT=lh, rhs=wkl[:, t, :],
                                         start=False, stop=False)
                        nc.tensor.matmul(out=vacc[mt][:, :], lhsT=lh, rhs=wv[:, t, :],
                                         start=first, stop=last)
                        # ll-stationary group
                        nc.tensor.matmul(out=qacc[mt][:, :], lhsT=ll, rhs=wqh[:, t, :],
                                         start=False, stop=last)
                        nc.tensor.matmul(out=kacc[mt][:, :], lhsT=ll, rhs=wkh[:, t, :],
                                         start=False, stop=last)

            # ---- k: split hi/lo f16 and store ----
            for mt in range(2):
                kf = sp.tile([128, OS], f32, tag=f"kf{mt}")
                nc.scalar.copy(out=kf[:, :], in_=kacc[mt][:, :])
                kh16 = sp.tile([128, OS], f16, tag=f"kh{mt}")
                nc.scalar.copy(out=kh16[:, :], in_=kf[:, :])
                kh32 = sp.tile([128, OS], f32, tag=f"kh32{mt}")
                nc.scalar.copy(out=kh32[:, :], in_=kh16[:, :])
                kl16 = sp.tile([128, OS], f16, tag=f"kl{mt}")
                nc.vector.tensor_sub(out=kl16[:, :], in0=kf[:, :], in1=kh32[:, :])
                nc.sync.dma_start(out=KH[mt * 128:(mt + 1) * 128, :], in_=kh16[:, :])
                nc.sync.dma_start(out=KL[mt * 128:(mt + 1) * 128, :], in_=kl16[:, :])

            # ---- v to f16; u tables: u[b,k] = (v_b^T @ cw_k) as 4 M-tiles ----
            vsb = {}
            for mt in range(2):
                v16 = sp.tile([128, OS], f16, tag=f"vsb{mt}")
                nc.scalar.copy(out=v16[:, :], in_=vacc[mt][:, :])
                vsb[mt] = v16
            for b in range(B):
                off = (b % 2) * C
                vt = vsb[b // 2][off:off + C, :]  # [64, 512] f16
                for k in range(K):
                    u16 = sp.tile([128, 4, C], f16, tag="u16")
                    for m in range(4):
                        pt = pu.tile([128, C], f32, tag="pu")
                        nc.tensor.matmul(out=pt[:, :],
                                         lhsT=vt[:, m * 128:(m + 1) * 128],
                                         rhs=cw[off:off + C, k * C:(k + 1) * C],
                                         start=True, stop=True)
                        nc.scalar.copy(out=u16[:, m, :], in_=pt[:, :])
                    for m in range(4):
                        nc.sync.dma_start(out=UO[b, k, m], in_=u16[:, m, :])

            # ---- qn = q / ||q||_col, emitted as hi/lo f16 ----
            for b in range(B):
                off = (b % 2) * C
                q_b = qacc[b // 2][off:off + C, :]  # [64, 512] f32 psum
                qf = sp.tile([128, OS], f32, tag="qf")
                nc.scalar.copy(out=qf[off:off + C, :], in_=q_b)
                sq = sp.tile([128, OS], f32, tag="sq")
                nc.scalar.square(out=sq[off:off + C, :], in_=qf[off:off + C, :])
                pscr = pu.tile([128, OS], f32, tag="pscr", name="pscr")
                nc.tensor.matmul(out=pscr[0:1, :], lhsT=ones_r[off:off + C, :],
                                 rhs=sq[off:off + C, :], start=True, stop=True)
                nrm = sp.tile([1, OS], f32, tag="nrm")
                nc.scalar.sqrt(out=nrm[:, :], in_=pscr[0:1, :])
                rcp = sp.tile([1, OS], f32, tag="rcp")
                nc.vector.reciprocal(out=rcp[:, :], in_=nrm[:, :])
                pb = pu.tile([128, OS], f32, tag="pscr", name="pb")
                nc.tensor.matmul(out=pb[off:off + C, :], lhsT=ones_b[:, :],
                                 rhs=rcp[:, :], start=True, stop=True)
                bc = sp.tile([128, OS], f32, tag="bc")
                nc.scalar.copy(out=bc[off:off + C, :], in_=pb[off:off + C, :])
                qn = sp.tile([128, OS], f32, tag="qn")
                nc.vector.tensor_mul(out=qn[off:off + C, :], in0=qf[off:off + C, :],
                                     in1=bc[off:off + C, :])
                qh16 = sp.tile([128, OS], f16, tag="qh16")
                nc.scalar.copy(out=qh16[off:off + C, :], in_=qn[off:off + C, :])
                qh32 = sp.tile([128, OS], f32, tag="qh32")
                nc.scalar.copy(out=qh32[off:off + C, :], in_=qh16[off:off + C, :])
                ql16 = sp.tile([128, OS], f16, tag="ql16")
                nc.vector.tensor_sub(out=ql16[off:off + C, :],
                                     in0=qn[off:off + C, :], in1=qh32[off:off + C, :])
                nc.sync.dma_start(out=QNH[b * C:(b + 1) * C, :], in_=qh16[off:off + C, :])
                nc.sync.dma_start(out=QNL[b * C:(b + 1) * C, :], in_=ql16[off:off + C, :])
    nc.compile()
    return nc


def _build_l2():
    """Phase 2, SPMD core j: rows t in [j*512,(j+1)*512) for all batches.
    Exact sim via fp16 split-3 matmuls in 4 psum chunks of 1024; per-chunk
    exact top-8 via max/max_index on the f32 copy; merge 32 candidates via
    max8 + find_index + one-hot decode; gather-sum u tables -> yT;
    partial out = yT^T @ WoT slice."""
    nc = bacc.Bacc("TRN2", target_bir_lowering=False, debug=False, num_devices=NCORES)
    QNH = nc.dram_tensor("qnh", [BC, T], f16, kind="ExternalInput").ap()
    QNL = nc.dram_tensor("qnl", [BC, T], f16, kind="ExternalInput").ap()
    KHJ = nc.dram_tensor("khj", [BC, OS], f16, kind="ExternalInput").ap()
    KLJ = nc.dram_tensor("klj", [BC, OS], f16, kind="ExternalInput").ap()
    WOT = nc.dram_tensor("wot", [OS, T], f16, kind="ExternalInput").ap()
    IOTA = nc.dram_tensor("iota32", [128, 32], u32, kind="ExternalInput").ap()
    UT = [[nc.dram_tensor(f"ut{b}_{k}", [T, C], f16, kind="ExternalInput").ap()
           for k in range(K)] for b in range(B)]
    OUT = nc.dram_tensor("out_o", [2, 128, T], f32, kind="ExternalOutput").ap()

    NCH = 4          # sim chunks per block
    CW_ = T // NCH   # 1024 columns per chunk

    with tile.TileContext(nc) as tc:
        with tc.tile_pool(name="qp", bufs=1) as qp, \
             tc.tile_pool(name="wp", bufs=1) as wp, \
             tc.tile_pool(name="sp", bufs=3) as sp, \
             tc.tile_pool(name="simp", bufs=6) as simp, \
             tc.tile_pool(name="yp", bufs=1) as yp, \
             tc.tile_pool(name="obp", bufs=2) as obp, \
             tc.tile_pool(name="pp", bufs=4, space="PSUM") as pp, \
             tc.tile_pool(name="po", bufs=2, space="PSUM") as po:
            kh, kl = {}, {}
            for mt in range(2):
                h16 = qp.tile([128, OS], f16, tag=f"kh{mt}")
                nc.sync.dma_start(out=h16[:, :], in_=KHJ[mt * 128:(mt + 1) * 128, :])
                l16 = qp.tile([128, OS], f16, tag=f"kl{mt}")
                nc.sync.dma_start(out=l16[:, :], in_=KLJ[mt * 128:(mt + 1) * 128, :])
                kh[mt], kl[mt] = h16, l16
            qh, ql = {}, {}
            for mt in range(2):
                h16 = qp.tile([128, T], f16, tag=f"qh{mt}")
                l16 = qp.tile([128, T], f16, tag=f"ql{mt}")
                for cc in range(4):
                    cs = slice(cc * 1024, (cc + 1) * 1024)
                    nc.sync.dma_start(out=h16[:, cs], in_=QNH[mt * 128:(mt + 1) * 128, cs])
                    nc.sync.dma_start(out=l16[:, cs], in_=QNL[mt * 128:(mt + 1) * 128, cs])
                qh[mt], ql[mt] = h16, l16
            wot = {}
            iota = qp.tile([128, 32], u32, tag="iota")
            nc.sync.dma_start(out=iota[:, :], in_=IOTA[:, :])

            wu = qp.tile([128, 512], f16, tag="wu")
            nc.vector.memset(wu[:, :], 0.0)
            for _w in range(8):
                pw = po.tile([128, CW_], f32, tag="po", name=f"warm{_w}")
                nc.tensor.matmul(out=pw[:, :512], lhsT=wu[:, :128], rhs=wu[:, :],
                                 start=True, stop=True)

            ytp = {}  # (pair, kt) -> [128, 128] f16 lhsT tiles for the out matmul
            for pair in range(2):
                for kt in range(4):
                    yt_tile = yp.tile([128, 128], f16, tag=f"yt{pair}{kt}")
                    ytp[(pair, kt)] = yt_tile

            for b in range(B):
                khb = kh[b // 2][(b % 2) * C:(b % 2) * C + C, :]   # [64, 512] f16
                klb = kl[b // 2][(b % 2) * C:(b % 2) * C + C, :]
                qhb = qh[b // 2][(b % 2) * C:(b % 2) * C + C, :]   # [64, 4096] f16
                qlb = ql[b // 2][(b % 2) * C:(b % 2) * C + C, :]
                for i in range(4):  # 128-token blocks of this core's slice
                    lh = khb[:, i * 128:(i + 1) * 128]
                    ll = klb[:, i * 128:(i + 1) * 128]
                    simf = simp.tile([128, T], f32, tag="simf")
                    for ch in range(NCH):
                        for half in range(2):
                            c0 = ch * CW_ + half * 512
                            rh = qhb[:, c0:c0 + 512]
                            rl = qlb[:, c0:c0 + 512]
                            ps = pp.tile([128, 512], f32, tag="ps")
                            nc.tensor.matmul(out=ps[:, :], lhsT=lh, rhs=rh,
                                             start=True, stop=False)
                            nc.tensor.matmul(out=ps[:, :], lhsT=ll, rhs=rh,
                                             start=False, stop=False)
                            nc.tensor.matmul(out=ps[:, :], lhsT=lh, rhs=rl,
                                             start=False, stop=True)
                            nc.scalar.copy(out=simf[:, c0:c0 + 512], in_=ps[:, :])
                    # one full-row top-8 scan + one full-row index scan
                    g8 = sp.tile([128, 8], f32, tag="g8")
                    nc.vector.max(out=g8[:, :], in_=simf[:, :])
                    idx8 = sp.tile([128, 8], u32, tag="idx8")
                    nc.vector.max_index(out=idx8[:, :], in_max=g8[:, :],
                                        in_values=simf[:, :])
                    gth = sp.tile([128, K, C], f16, tag="gth")
                    for k in range(K):
                        nc.gpsimd.indirect_dma_start(
                            out=gth[:, k, :], out_offset=None,
                            in_=UT[b][k][:, :],
                            in_offset=bass.IndirectOffsetOnAxis(ap=idx8[:, k:k + 1], axis=0))
                    t0 = sp.tile([128, C], f16, tag="t0")
                    t1 = sp.tile([128, C], f16, tag="t1")
                    nc.gpsimd.tensor_add(out=t0[:, :], in0=gth[:, 0, :], in1=gth[:, 1, :])
                    nc.gpsimd.tensor_add(out=t1[:, :], in0=gth[:, 2, :], in1=gth[:, 3, :])
                    dst = ytp[(b // 2, i)][:, (b % 2) * C:(b % 2) * C + C]
                    nc.gpsimd.tensor_add(out=dst, in0=t0[:, :], in1=t1[:, :])

                # emit the pair's out-projection as soon as both batches done
                if b % 2 == 1:
                    pair = b // 2
                    if pair == 1:
                        # keep PE warm through the tail (no-dep matmuls)
                        for _w in range(10):
                            pw = pp.tile([128, 512], f32, tag="ps")
                            nc.tensor.matmul(out=pw[:, :], lhsT=wu[:, :128],
                                             rhs=wu[:, :], start=True, stop=True)
                    if not wot:
                        for kt in range(4):
                            w = wp.tile([128, T], f16, tag=f"wot{kt}", name=f"wot{kt}")
                            nc.sync.dma_start(out=w[:, :], in_=WOT[kt * 128:(kt + 1) * 128, :])
                            wot[kt] = w
                    ob = obp.tile([128, T], f32, tag="ob")
                    for ch in range(NCH):
                        ps = po.tile([128, CW_], f32, tag="po")
                        for half in range(2):
                            c0 = ch * CW_ + half * 512
                            po_s = ps[:, half * 512:(half + 1) * 512]
                            for kt in range(4):
                                nc.tensor.matmul(out=po_s, lhsT=ytp[(pair, kt)][:, :],
                                                 rhs=wot[kt][:, c0:c0 + 512],
                                                 start=(kt == 0), stop=(kt == 3))
                        nc.scalar.copy(out=ob[:, ch * CW_:(ch + 1) * CW_], in_=ps[:, :])
                        nc.sync.dma_start(out=OUT[pair, :, ch * CW_:(ch + 1) * CW_],
                                          in_=ob[:, ch * CW_:(ch + 1) * CW_])


    nc.compile()
    return nc


def _split16(a):
    h = a.astype(np.float16)
    l = (a - h.astype(np.float32)).astype(np.float16)
    return h, l


def kernel(x, Wq, Wk, Wv, Wo, conv_w, conv_b):
    x = np.asarray(x, np.float32)
    Wq = np.asarray(Wq, np.float32); Wk = np.asarray(Wk, np.float32)
    Wv = np.asarray(Wv, np.float32); Wo = np.asarray(Wo, np.float32)
    conv_w = np.asarray(conv_w, np.float32); conv_b = np.asarray(conv_b, np.float32)

    LAST_EXEC_NS.clear(); LAST_TRACE_DIRS.clear()
    if "l1" not in _cache:
        _cache["l1"] = _build_l1()
    if "l2" not in _cache:
        _cache["l2"] = _build_l2()

    def _sw(a):
        # [T, W] -> [128, T//128, W] with [p, kt, w] = a[kt*128+p, w]
        return np.ascontiguousarray(a.reshape(T // 128, 128, -1).transpose(1, 0, 2))

    xT = np.ascontiguousarray(x.transpose(2, 0, 1).reshape(T, BC))  # [t, b*64+c]
    xh, xl = _split16(xT)
    xh, xl = _sw(xh), _sw(xl)
    WqT, WkT = Wq.T, Wk.T
    WvT16 = np.ascontiguousarray(Wv.T).astype(np.float16)
    cw1 = np.ascontiguousarray(conv_w.transpose(1, 2, 0).reshape(C, K * C)).astype(np.float16)
    cw = np.concatenate([cw1, cw1], axis=0)
    # cw[ci, k*64+co] = conv_w[co, ci, k]

    in_maps = []
    for j in range(NCORES):
        sl = slice(j * OS, (j + 1) * OS)
        wqh, wql = _split16(np.ascontiguousarray(WqT[:, sl]))
        wkh, wkl = _split16(np.ascontiguousarray(WkT[:, sl]))
        in_maps.append({"xh": xh, "xl": xl,
                        "wqh": _sw(wqh), "wql": _sw(wql),
                        "wkh": _sw(wkh), "wkl": _sw(wkl),
                        "wv": _sw(np.ascontiguousarray(WvT16[:, sl])), "cw": cw})
    r1 = _run(_cache["l1"], in_maps)

    qnh = np.concatenate([r1.results[j]["qnh_o"] for j in range(NCORES)], axis=1)
    qnl = np.concatenate([r1.results[j]["qnl_o"] for j in range(NCORES)], axis=1)
    ut = {}
    for b in range(B):
        for k in range(K):
            ut[(b, k)] = np.ascontiguousarray(np.concatenate(
                [r1.results[j]["u_o"][b, k].reshape(OS, C) for j in range(NCORES)], axis=0))

    iota32 = np.broadcast_to(np.arange(32, dtype=np.uint32), (128, 32)).copy()
    in_maps2 = []
    for j in range(NCORES):
        m = {"qnh": qnh, "qnl": qnl,
             "khj": r1.results[j]["kh_o"], "klj": r1.results[j]["kl_o"],
             "wot": np.ascontiguousarray(Wo.T[j * OS:(j + 1) * OS, :]).astype(np.float16),
             "iota32": iota32}
        for b in range(B):
            for k in range(K):
                m[f"ut{b}_{k}"] = ut[(b, k)]
        in_maps2.append(m)
    r2 = _run(_cache["l2"], in_maps2)

    out = np.zeros((B, C, T), np.float32)
    for j in range(NCORES):
        oo = r2.results[j]["out_o"]  # [2, 128, T]
        for b in range(B):
            out[b] += oo[b // 2, (b % 2) * C:(b % 2) * C + C, :]
    bias = conv_b[:, None] * Wo.sum(axis=1)[None, :]  # [64, 4096]
    out += bias[None, :, :]
    return out


# revision 39
# speedup vs baseline: 1.1079x; 1.0021x over previous
import sys
for p in ('/opt/trn_rl_repo', '/opt/pypackages'):
    if p not in sys.path:
        sys.path.insert(0, p)
import numpy as np
from concourse import bass, bacc, tile, mybir
from concourse import bass_utils

B, C, T, K = 4, 64, 4096, 4
NCORES = 8
OS = T // NCORES          # 512: per-core token slice (phase-1 o-slice == phase-2 t-slice)
BC = B * C                # 256
f32 = mybir.dt.float32
f16 = mybir.dt.float16
u32 = mybir.dt.uint32

_cache = {}
LAST_EXEC_NS = []
LAST_TRACE_DIRS = []


def _run(nc, in_maps):
    r = bass_utils.run_bass_kernel_spmd(nc, in_maps, core_ids=list(range(NCORES)))
    LAST_EXEC_NS.append(getattr(r, "exec_time_ns", None))
    it = getattr(r, "instructions_and_trace", None)
    if it is not None:
        LAST_TRACE_DIRS.append(it[1])
    return r


def _build_l1():
    """Phase 1, SPMD core j: single kt-sweep computing q,k (fp16 split-3) and v
    (fp16) o-slices with grouped stationary operands; emits qn hi/lo f16,
    k hi/lo f16 and the conv-folded value tables u."""
    nc = bacc.Bacc("TRN2", target_bir_lowering=False, debug=False, num_devices=NCORES)
    XH = nc.dram_tensor("xh", [128, T // 128, BC], f16, kind="ExternalInput").ap()
    XL = nc.dram_tensor("xl", [128, T // 128, BC], f16, kind="ExternalInput").ap()
    WQH = nc.dram_tensor("wqh", [128, T // 128, OS], f16, kind="ExternalInput").ap()
    WQL = nc.dram_tensor("wql", [128, T // 128, OS], f16, kind="ExternalInput").ap()
    WKH = nc.dram_tensor("wkh", [128, T // 128, OS], f16, kind="ExternalInput").ap()
    WKL = nc.dram_tensor("wkl", [128, T // 128, OS], f16, kind="ExternalInput").ap()
    WV = nc.dram_tensor("wv", [128, T // 128, OS], f16, kind="ExternalInput").ap()
    CW = nc.dram_tensor("cw", [2 * C, K * C], f16, kind="ExternalInput").ap()
    QNH = nc.dram_tensor("qnh_o", [BC, OS], f16, kind="ExternalOutput").ap()
    QNL = nc.dram_tensor("qnl_o", [BC, OS], f16, kind="ExternalOutput").ap()
    KH = nc.dram_tensor("kh_o", [BC, OS], f16, kind="ExternalOutput").ap()
    KL = nc.dram_tensor("kl_o", [BC, OS], f16, kind="ExternalOutput").ap()
    UO = nc.dram_tensor("u_o", [B, K, 128, 4, C], f16, kind="ExternalOutput").ap()

    NKT = T // 128  # 32 contraction tiles
    NHF = 16
    H = NKT // NHF  # 8 kt per weight chunk

    with tile.TileContext(nc) as tc:
        with tc.tile_pool(name="xp", bufs=1) as xp, \
             tc.tile_pool(name="wp", bufs=4) as wp, \
             tc.tile_pool(name="sp", bufs=2) as sp, \
             tc.tile_pool(name="cp", bufs=1) as cp, \
             tc.tile_pool(name="pp", bufs=1, space="PSUM") as pp, \
             tc.tile_pool(name="pu", bufs=1, space="PSUM") as pu:
            xh = xp.tile([128, NKT, BC], f16, tag="xh")
            xl = xp.tile([128, NKT, BC], f16, tag="xl")
            cw = cp.tile([2 * C, K * C], f16, tag="cw")
            nc.sync.dma_start(out=cw[:, :], in_=CW[:, :])
            ones_r = cp.tile([128, 1], f32, tag="ones_r")   # reduce lhsT
            nc.vector.memset(ones_r[:, :], 1.0)
            ones_b = cp.tile([1, C], f32, tag="ones_b")   # broadcast lhsT
            nc.vector.memset(ones_b[:, :], 1.0)

            wu1 = cp.tile([128, 512], f16, tag="wu1")
            nc.vector.memset(wu1[:, :], 0.0)
            for _w in range(6):
                pw = pu.tile([128, OS], f32, tag="pscr", name=f"warm{_w}")
                nc.tensor.matmul(out=pw[:, :], lhsT=wu1[:, :128], rhs=wu1[:, :],
                                 start=True, stop=True)

            qacc, kacc, vacc = [], [], []
            for mt in range(2):
                qacc.append(pp.tile([128, OS], f32, tag=f"qa{mt}", name=f"qa{mt}"))
                kacc.append(pp.tile([128, OS], f32, tag=f"ka{mt}", name=f"ka{mt}"))
                vacc.append(pp.tile([128, OS], f32, tag=f"va{mt}", name=f"va{mt}"))

            for hf in range(NHF):
                slx = slice(hf * H, (hf + 1) * H)
                nc.sync.dma_start(out=xh[:, slx, :], in_=XH[:, slx, :])
                nc.sync.dma_start(out=xl[:, slx, :], in_=XL[:, slx, :])
                wqh = wp.tile([128, H, OS], f16, tag="wqh")
                wql = wp.tile([128, H, OS], f16, tag="wql")
                wkh = wp.tile([128, H, OS], f16, tag="wkh")
                wkl = wp.tile([128, H, OS], f16, tag="wkl")
                wv = wp.tile([128, H, OS], f16, tag="wv")
                sl = slice(hf * H, (hf + 1) * H)
                nc.sync.dma_start(out=wqh[:, :, :], in_=WQH[:, sl, :])
                nc.sync.dma_start(out=wql[:, :, :], in_=WQL[:, sl, :])
                nc.sync.dma_start(out=wkh[:, :, :], in_=WKH[:, sl, :])
                nc.sync.dma_start(out=wkl[:, :, :], in_=WKL[:, sl, :])
                nc.sync.dma_start(out=wv[:, :, :], in_=WV[:, sl, :])
                for t in range(H):
                    kt = hf * H + t
                    first = (kt == 0)
                    last = (kt == NKT - 1)
                    for mt in range(2):
                        lh = xh[:, kt, mt * 128:(mt + 1) * 128]
                        ll = xl[:, kt, mt * 128:(mt + 1) * 128]
                        # lh-stationary group
                        nc.tensor.matmul(out=qacc[mt][:, :], lhsT=lh, rhs=wqh[:, t, :],
                                         start=first, stop=False)
                        nc.tensor.matmul(out=qacc[mt][:, :], lhsT=lh, rhs=wql[:, t, :],
                                         start=False, stop=False)
                        nc.tensor.matmul(out=kacc[mt][:, :], lhsT=lh, rhs=wkh[:, t, :],
                                         start=first, stop=False)
                        nc.tensor.matmul(out=kacc[mt][:, :], lhsT=lh, rhs=wkl[:, t, :],
                                         start=False, stop=False)
                        nc.tensor.matmul(out=vacc[mt][:, :], lhsT=lh, rhs=wv[:, t, :],
                                         start=first, stop=last)
                        # ll-stationary group
                        nc.tensor.matmul(out=qacc[mt][:, :], lhsT=ll, rhs=wqh[:, t, :],
                                         start=False, stop=last)
                        nc.tensor.matmul(out=kacc[mt][:, :], lhsT=ll, rhs=wkh[:, t, :],
                                         start=False, stop=last)

            # ---- k: split hi/lo f16 and store ----
            for mt in range(2):
                kf = sp.tile([128, OS], f32, tag=f"kf{mt}")
                nc.scalar.copy(out=kf[:, :], in_=kacc[mt][:, :])
                kh16 = sp.tile([128, OS], f16, tag=f"kh{mt}")
                nc.scalar.copy(out=kh16[:, :], in_=kf[:, :])
                kh32 = sp.tile([128, OS], f32, tag=f"kh32{mt}")
                nc.scalar.copy(out=kh32[:, :], in_=kh16[:, :])
                kl16 = sp.tile([128, OS], f16, tag=f"kl{mt}")
                nc.vector.tensor_sub(out=kl16[:, :], in0=kf[:, :], in1=kh32[:, :])
                nc.sync.dma_start(out=KH[mt * 128:(mt + 1) * 128, :], in_=kh16[:, :])
                nc.sync.dma_start(out=KL[mt * 128:(mt + 1) * 128, :], in_=kl16[:, :])

            # ---- v to f16; u tables: u[b,k] = (v_b^T @ cw_k) as 4 M-tiles ----
            vsb = {}
            for mt in range(2):
                v16 = sp.tile([128, OS], f16, tag=f"vsb{mt}")
                nc.scalar.copy(out=v16[:, :], in_=vacc[mt][:, :])
                vsb[mt] = v16
            for b in range(B):
                off = (b % 2) * C
                vt = vsb[b // 2][off:off + C, :]  # [64, 512] f16
                for k in range(K):
                    u16 = sp.tile([128, 4, C], f16, tag="u16")
                    for m in range(4):
                        pt = pu.tile([128, C], f32, tag="pu")
                        nc.tensor.matmul(out=pt[:, :],
                                         lhsT=vt[:, m * 128:(m + 1) * 128],
                                         rhs=cw[off:off + C, k * C:(k + 1) * C],
                                         start=True, stop=True)
                        nc.scalar.copy(out=u16[:, m, :], in_=pt[:, :])
                    nc.sync.dma_start(out=UO[b, k], in_=u16[:, :, :])

            # ---- qn = q / ||q||_col, emitted as hi/lo f16 ----
            for b in range(B):
                off = (b % 2) * C
                q_b = qacc[b // 2][off:off + C, :]  # [64, 512] f32 psum
                qf = sp.tile([128, OS], f32, tag="qf")
                nc.scalar.copy(out=qf[off:off + C, :], in_=q_b)
                sq = sp.tile([128, OS], f32, tag="sq")
                nc.scalar.square(out=sq[off:off + C, :], in_=qf[off:off + C, :])
                pscr = pu.tile([128, OS], f32, tag="pscr", name="pscr")
                nc.tensor.matmul(out=pscr[0:1, :], lhsT=ones_r[off:off + C, :],
                                 rhs=sq[off:off + C, :], start=True, stop=True)
                nrm = sp.tile([1, OS], f32, tag="nrm")
                nc.scalar.sqrt(out=nrm[:, :], in_=pscr[0:1, :])
                rcp = sp.tile([1, OS], f32, tag="rcp")
                nc.vector.reciprocal(out=rcp[:, :], in_=nrm[:, :])
                pb = pu.tile([128, OS], f32, tag="pscr", name="pb")
                nc.tensor.matmul(out=pb[off:off + C, :], lhsT=ones_b[:, :],
                                 rhs=rcp[:, :], start=True, stop=True)
                bc = sp.tile([128, OS], f32, tag="bc")
                nc.scalar.copy(out=bc[off:off + C, :], in_=pb[off:off + C, :])
                qn = sp.tile([128, OS], f32, tag="qn")
                nc.vector.tensor_mul(out=qn[off:off + C, :], in0=qf[off:off + C, :],
                                     in1=bc[off:off + C, :])
                qh16 = sp.tile([128, OS], f16, tag="qh16")
                nc.scalar.copy(out=qh16[off:off + C, :], in_=qn[off:off + C, :])
                qh32 = sp.tile([128, OS], f32, tag="qh32")
                nc.scalar.copy(out=qh32[off:off + C, :], in_=qh16[off:off + C, :])
                ql16 = sp.tile([128, OS], f16, tag="ql16")
                nc.vector.tensor_sub(out=ql16[off:off + C, :],
                                     in0=qn[off:off + C, :], in1=qh32[off:off + C, :])
                nc.sync.dma_start(out=QNH[b * C:(b + 1) * C, :], in_=qh16[off:off + C, :])
                nc.sync.dma_start(out=QNL[b * C:(b + 1) * C, :], in_=ql16[off:off + C, :])
    nc.compile()
    return nc


def _build_l2():
    """Phase 2, SPMD core j: rows t in [j*512,(j+1)*512) for all batches.
    Exact sim via fp16 split-3 matmuls in 4 psum chunks of 1024; per-chunk
    exact top-8 via max/max_index on the f32 copy; merge 32 candidates via
    max8 + find_index + one-hot decode; gather-sum u tables -> yT;
    partial out = yT^T @ WoT slice."""
    nc = bacc.Bacc("TRN2", target_bir_lowering=False, debug=False, num_devices=NCORES)
    QNH = nc.dram_tensor("qnh", [BC, T], f16, kind="ExternalInput").ap()
    QNL = nc.dram_tensor("qnl", [BC, T], f16, kind="ExternalInput").ap()
    KHJ = nc.dram_tensor("khj", [BC, OS], f16, kind="ExternalInput").ap()
    KLJ = nc.dram_tensor("klj", [BC, OS], f16, kind="ExternalInput").ap()
    WOT = nc.dram_tensor("wot", [OS, T], f16, kind="ExternalInput").ap()
    IOTA = nc.dram_tensor("iota32", [128, 32], u32, kind="ExternalInput").ap()
    UT = [[nc.dram_tensor(f"ut{b}_{k}", [T, C], f16, kind="ExternalInput").ap()
           for k in range(K)] for b in range(B)]
    OUT = nc.dram_tensor("out_o", [2, 128, T], f32, kind="ExternalOutput").ap()

    NCH = 4          # sim chunks per block
    CW_ = T // NCH   # 1024 columns per chunk

    with tile.TileContext(nc) as tc:
        with tc.tile_pool(name="qp", bufs=1) as qp, \
             tc.tile_pool(name="wp", bufs=1) as wp, \
             tc.tile_pool(name="sp", bufs=3) as sp, \
             tc.tile_pool(name="simp", bufs=6) as simp, \
             tc.tile_pool(name="yp", bufs=1) as yp, \
             tc.tile_pool(name="obp", bufs=2) as obp, \
             tc.tile_pool(name="pp", bufs=4, space="PSUM") as pp, \
             tc.tile_pool(name="po", bufs=2, space="PSUM") as po:
            kh, kl = {}, {}
            for mt in range(2):
                h16 = qp.tile([128, OS], f16, tag=f"kh{mt}")
                nc.sync.dma_start(out=h16[:, :], in_=KHJ[mt * 128:(mt + 1) * 128, :])
                l16 = qp.tile([128, OS], f16, tag=f"kl{mt}")
                nc.sync.dma_start(out=l16[:, :], in_=KLJ[mt * 128:(mt + 1) * 128, :])
                kh[mt], kl[mt] = h16, l16
            qh, ql = {}, {}
            for mt in range(2):
                h16 = qp.tile([128, T], f16, tag=f"qh{mt}")
                l16 = qp.tile([128, T], f16, tag=f"ql{mt}")
                for cc in range(4):
                    cs = slice(cc * 1024, (cc + 1) * 1024)
                    nc.sync.dma_start(out=h16[:, cs], in_=QNH[mt * 128:(mt + 1) * 128, cs])
                    nc.sync.dma_start(out=l16[:, cs], in_=QNL[mt * 128:(mt + 1) * 128, cs])
                qh[mt], ql[mt] = h16, l16
            wot = {}
            iota = qp.tile([128, 32], u32, tag="iota")
            nc.sync.dma_start(out=iota[:, :], in_=IOTA[:, :])

            wu = qp.tile([128, 512], f16, tag="wu")
            nc.vector.memset(wu[:, :], 0.0)
            for _w in range(8):
                pw = po.tile([128, CW_], f32, tag="po", name=f"warm{_w}")
                nc.tensor.matmul(out=pw[:, :512], lhsT=wu[:, :128], rhs=wu[:, :],
                                 start=True, stop=True)

            ytp = {}  # (pair, kt) -> [128, 128] f16 lhsT tiles for the out matmul
            for pair in range(2):
                for kt in range(4):
                    yt_tile = yp.tile([128, 128], f16, tag=f"yt{pair}{kt}")
                    ytp[(pair, kt)] = yt_tile

            for b in range(B):
                khb = kh[b // 2][(b % 2) * C:(b % 2) * C + C, :]   # [64, 512] f16
                klb = kl[b // 2][(b % 2) * C:(b % 2) * C + C, :]
                qhb = qh[b // 2][(b % 2) * C:(b % 2) * C + C, :]   # [64, 4096] f16
                qlb = ql[b // 2][(b % 2) * C:(b % 2) * C + C, :]
                for i in range(4):  # 128-token blocks of this core's slice
                    lh = khb[:, i * 128:(i + 1) * 128]
                    ll = klb[:, i * 128:(i + 1) * 128]
                    simf = simp.tile([128, T], f32, tag="simf")
                    for ch in range(NCH):
                        for half in range(2):
                            c0 = ch * CW_ + half * 512
                            rh = qhb[:, c0:c0 + 512]
                            rl = qlb[:, c0:c0 + 512]
                            ps = pp.tile([128, 512], f32, tag="ps")
                            nc.tensor.matmul(out=ps[:, :], lhsT=lh, rhs=rh,
                                             start=True, stop=False)
                            nc.tensor.matmul(out=ps[:, :], lhsT=ll, rhs=rh,
                                             start=False, stop=False)
                            nc.tensor.matmul(out=ps[:, :], lhsT=lh, rhs=rl,
                                             start=False, stop=True)
                            nc.scalar.copy(out=simf[:, c0:c0 + 512], in_=ps[:, :])
                    # one full-row top-8 scan + one full-row index scan
                    g8 = sp.tile([128, 8], f32, tag="g8")
                    nc.vector.max(out=g8[:, :], in_=simf[:, :])
                    idx8 = sp.tile([128, 8], u32, tag="idx8")
                    nc.vector.max_index(out=idx8[:, :], in_max=g8[:, :],
                                        in_values=simf[:, :])
                    gth = sp.tile([128, K, C], f16, tag="gth")
                    for k in range(K):
                        nc.gpsimd.indirect_dma_start(
                            out=gth[:, k, :], out_offset=None,
                            in_=UT[b][k][:, :],
                            in_offset=bass.IndirectOffsetOnAxis(ap=idx8[:, k:k + 1], axis=0))
                    t0 = sp.tile([128, C], f16, tag="t0")
                    t1 = sp.tile([128, C], f16, tag="t1")
                    nc.gpsimd.tensor_add(out=t0[:, :], in0=gth[:, 0, :], in1=gth[:, 1, :])
                    nc.gpsimd.tensor_add(out=t1[:, :], in0=gth[:, 2, :], in1=gth[:, 3, :])
                    dst = ytp[(b // 2, i)][:, (b % 2) * C:(b % 2) * C + C]
                    nc.gpsimd.tensor_add(out=dst, in0=t0[:, :], in1=t1[:, :])

                # emit the pair's out-projection as soon as both batches done
                if b % 2 == 1:
                    pair = b // 2
                    if pair == 1:
                        # keep PE warm through the tail (no-dep matmuls)
                        for _w in range(10):
                            pw = pp.tile([128, 512], f32, tag="ps")
                            nc.tensor.matmul(out=pw[:, :], lhsT=wu[:, :128],
                                             rhs=wu[:, :], start=True, stop=True)
                    if not wot:
                        for kt in range(4):
                            w = wp.tile([128, T], f16, tag=f"wot{kt}", name=f"wot{kt}")
                            nc.sync.dma_start(out=w[:, :], in_=WOT[kt * 128:(kt + 1) * 128, :])
                            wot[kt] = w
                    ob = obp.tile([128, T], f32, tag="ob")
                    for ch in range(NCH):
                        ps = po.tile([128, CW_], f32, tag="po")
                        for half in range(2):
                            c0 = ch * CW_ + half * 512
                            po_s = ps[:, half * 512:(half + 1) * 512]
                            for kt in range(4):
                                nc.tensor.matmul(out=po_s, lhsT=ytp[(pair, kt)][:, :],
                                                 rhs=wot[kt][:, c0:c0 + 512],
                                                 start=(kt == 0), stop=(kt == 3))
                        nc.scalar.copy(out=ob[:, ch * CW_:(ch + 1) * CW_], in_=ps[:, :])
                        nc.sync.dma_start(out=OUT[pair, :, ch * CW_:(ch + 1) * CW_],
                                          in_=ob[:, ch * CW_:(ch + 1) * CW_])


    nc.compile()
    return nc


def _split16(a):
    h = a.astype(np.float16)
    l = (a - h.astype(np.float32)).astype(np.float16)
    return h, l


def kernel(x, Wq, Wk, Wv, Wo, conv_w, conv_b):
    x = np.asarray(x, np.float32)
    Wq = np.asarray(Wq, np.float32); Wk = np.asarray(Wk, np.float32)
    Wv = np.asarray(Wv, np.float32); Wo = np.asarray(Wo, np.float32)
    conv_w = np.asarray(conv_w, np.float32); conv_b = np.asarray(conv_b, np.float32)

    LAST_EXEC_NS.clear(); LAST_TRACE_DIRS.clear()
    if "l1" not in _cache:
        _cache["l1"] = _build_l1()
    if "l2" not in _cache:
        _cache["l2"] = _build_l2()

    def _sw(a):
        # [T, W] -> [128, T//128, W] with [p, kt, w] = a[kt*128+p, w]
        return np.ascontiguousarray(a.reshape(T // 128, 128, -1).transpose(1, 0, 2))

    xT = np.ascontiguousarray(x.transpose(2, 0, 1).reshape(T, BC))  # [t, b*64+c]
    xh, xl = _split16(xT)
    xh, xl = _sw(xh), _sw(xl)
    WqT, WkT = Wq.T, Wk.T
    WvT16 = np.ascontiguousarray(Wv.T).astype(np.float16)
    cw1 = np.ascontiguousarray(conv_w.transpose(1, 2, 0).reshape(C, K * C)).astype(np.float16)
    cw = np.concatenate([cw1, cw1], axis=0)
    # cw[ci, k*64+co] = conv_w[co, ci, k]

    in_maps = []
    for j in range(NCORES):
        sl = slice(j * OS, (j + 1) * OS)
        wqh, wql = _split16(np.ascontiguousarray(WqT[:, sl]))
        wkh, wkl = _split16(np.ascontiguousarray(WkT[:, sl]))
        in_maps.append({"xh": xh, "xl": xl,
                        "wqh": _sw(wqh), "wql": _sw(wql),
                        "wkh": _sw(wkh), "wkl": _sw(wkl),
                        "wv": _sw(np.ascontiguousarray(WvT16[:, sl])), "cw": cw})
    r1 = _run(_cache["l1"], in_maps)

    qnh = np.concatenate([r1.results[j]["qnh_o"] for j in range(NCORES)], axis=1)
    qnl = np.concatenate([r1.results[j]["qnl_o"] for j in range(NCORES)], axis=1)
    ut = {}
    for b in range(B):
        for k in range(K):
            ut[(b, k)] = np.ascontiguousarray(np.concatenate(
                [r1.results[j]["u_o"][b, k].transpose(1, 0, 2).reshape(OS, C)
                 for j in range(NCORES)], axis=0))

    iota32 = np.broadcast_to(np.arange(32, dtype=np.uint32), (128, 32)).copy()
    in_maps2 = []
    for j in range(NCORES):
        m = {"qnh": qnh, "qnl": qnl,
             "khj": r1.results[j]["kh_o"], "klj": r1.results[j]["kl_o"],
             "wot": np.ascontiguousarray(Wo.T[j * OS:(j + 1) * OS, :]).astype(np.float16),
             "iota32": iota32}
        for b in range(B):
            for k in range(K):
                m[f"ut{b}_{k}"] = ut[(b, k)]
        in_maps2.append(m)
    r2 = _run(_cache["l2"], in_maps2)

    out = np.zeros((B, C, T), np.float32)
    for j in range(NCORES):
        oo = r2.results[j]["out_o"]  # [2, 128, T]
        for b in range(B):
            out[b] += oo[b // 2, (b % 2) * C:(b % 2) * C + C, :]
    bias = conv_b[:, None] * Wo.sum(axis=1)[None, :]  # [64, 4096]
    out += bias[None, :, :]
    return out


# revision 40
# speedup vs baseline: 1.1104x; 1.0023x over previous
import sys
for p in ('/opt/trn_rl_repo', '/opt/pypackages'):
    if p not in sys.path:
        sys.path.insert(0, p)
import numpy as np
from concourse import bass, bacc, tile, mybir
from concourse import bass_utils

B, C, T, K = 4, 64, 4096, 4
NCORES = 8
OS = T // NCORES          # 512: per-core token slice (phase-1 o-slice == phase-2 t-slice)
BC = B * C                # 256
f32 = mybir.dt.float32
f16 = mybir.dt.float16
u32 = mybir.dt.uint32

_cache = {}
LAST_EXEC_NS = []
LAST_TRACE_DIRS = []


def _run(nc, in_maps):
    r = bass_utils.run_bass_kernel_spmd(nc, in_maps, core_ids=list(range(NCORES)))
    LAST_EXEC_NS.append(getattr(r, "exec_time_ns", None))
    it = getattr(r, "instructions_and_trace", None)
    if it is not None:
        LAST_TRACE_DIRS.append(it[1])
    return r


def _build_l1():
    """Phase 1, SPMD core j: single kt-sweep computing q,k (fp16 split-3) and v
    (fp16) o-slices with grouped stationary operands; emits qn hi/lo f16,
    k hi/lo f16 and the conv-folded value tables u."""
    nc = bacc.Bacc("TRN2", target_bir_lowering=False, debug=False, num_devices=NCORES)
    XH = nc.dram_tensor("xh", [128, T // 128, BC], f16, kind="ExternalInput").ap()
    XL = nc.dram_tensor("xl", [128, T // 128, BC], f16, kind="ExternalInput").ap()
    WQH = nc.dram_tensor("wqh", [128, T // 128, OS], f16, kind="ExternalInput").ap()
    WQL = nc.dram_tensor("wql", [128, T // 128, OS], f16, kind="ExternalInput").ap()
    WKH = nc.dram_tensor("wkh", [128, T // 128, OS], f16, kind="ExternalInput").ap()
    WKL = nc.dram_tensor("wkl", [128, T // 128, OS], f16, kind="ExternalInput").ap()
    WV = nc.dram_tensor("wv", [128, T // 128, OS], f16, kind="ExternalInput").ap()
    CW = nc.dram_tensor("cw", [2 * C, K * C], f16, kind="ExternalInput").ap()
    QNH = nc.dram_tensor("qnh_o", [BC, OS], f16, kind="ExternalOutput").ap()
    QNL = nc.dram_tensor("qnl_o", [BC, OS], f16, kind="ExternalOutput").ap()
    KH = nc.dram_tensor("kh_o", [BC, OS], f16, kind="ExternalOutput").ap()
    KL = nc.dram_tensor("kl_o", [BC, OS], f16, kind="ExternalOutput").ap()
    UO = nc.dram_tensor("u_o", [B, K, 128, 4, C], f16, kind="ExternalOutput").ap()

    NKT = T // 128  # 32 contraction tiles
    NHF = 16
    H = NKT // NHF  # 8 kt per weight chunk

    with tile.TileContext(nc) as tc:
        with tc.tile_pool(name="xp", bufs=1) as xp, \
             tc.tile_pool(name="wp", bufs=4) as wp, \
             tc.tile_pool(name="sp", bufs=3) as sp, \
             tc.tile_pool(name="cp", bufs=1) as cp, \
             tc.tile_pool(name="pp", bufs=1, space="PSUM") as pp, \
             tc.tile_pool(name="pu", bufs=1, space="PSUM") as pu:
            xh = xp.tile([128, NKT, BC], f16, tag="xh")
            xl = xp.tile([128, NKT, BC], f16, tag="xl")
            cw = cp.tile([2 * C, K * C], f16, tag="cw")
            nc.sync.dma_start(out=cw[:, :], in_=CW[:, :])
            ones_r = cp.tile([128, 1], f32, tag="ones_r")   # reduce lhsT
            nc.vector.memset(ones_r[:, :], 1.0)
            ones_b = cp.tile([1, C], f32, tag="ones_b")   # broadcast lhsT
            nc.vector.memset(ones_b[:, :], 1.0)

            wu1 = cp.tile([128, 512], f16, tag="wu1")
            nc.vector.memset(wu1[:, :], 0.0)
            for _w in range(6):
                pw = pu.tile([128, OS], f32, tag="pscr", name=f"warm{_w}")
                nc.tensor.matmul(out=pw[:, :], lhsT=wu1[:, :128], rhs=wu1[:, :],
                                 start=True, stop=True)

            qacc, kacc, vacc = [], [], []
            for mt in range(2):
                qacc.append(pp.tile([128, OS], f32, tag=f"qa{mt}", name=f"qa{mt}"))
                kacc.append(pp.tile([128, OS], f32, tag=f"ka{mt}", name=f"ka{mt}"))
                vacc.append(pp.tile([128, OS], f32, tag=f"va{mt}", name=f"va{mt}"))

            for hf in range(NHF):
                slx = slice(hf * H, (hf + 1) * H)
                nc.sync.dma_start(out=xh[:, slx, :], in_=XH[:, slx, :])
                nc.sync.dma_start(out=xl[:, slx, :], in_=XL[:, slx, :])
                wqh = wp.tile([128, H, OS], f16, tag="wqh")
                wql = wp.tile([128, H, OS], f16, tag="wql")
                wkh = wp.tile([128, H, OS], f16, tag="wkh")
                wkl = wp.tile([128, H, OS], f16, tag="wkl")
                wv = wp.tile([128, H, OS], f16, tag="wv")
                sl = slice(hf * H, (hf + 1) * H)
                nc.sync.dma_start(out=wqh[:, :, :], in_=WQH[:, sl, :])
                nc.sync.dma_start(out=wql[:, :, :], in_=WQL[:, sl, :])
                nc.sync.dma_start(out=wkh[:, :, :], in_=WKH[:, sl, :])
                nc.sync.dma_start(out=wkl[:, :, :], in_=WKL[:, sl, :])
                nc.sync.dma_start(out=wv[:, :, :], in_=WV[:, sl, :])
                for t in range(H):
                    kt = hf * H + t
                    first = (kt == 0)
                    last = (kt == NKT - 1)
                    for mt in range(2):
                        lh = xh[:, kt, mt * 128:(mt + 1) * 128]
                        ll = xl[:, kt, mt * 128:(mt + 1) * 128]
                        # lh-stationary group
                        nc.tensor.matmul(out=qacc[mt][:, :], lhsT=lh, rhs=wqh[:, t, :],
                                         start=first, stop=False)
                        nc.tensor.matmul(out=qacc[mt][:, :], lhsT=lh, rhs=wql[:, t, :],
                                         start=False, stop=False)
                        nc.tensor.matmul(out=kacc[mt][:, :], lhsT=lh, rhs=wkh[:, t, :],
                                         start=first, stop=False)
                        nc.tensor.matmul(out=kacc[mt][:, :], lhsT=lh, rhs=wkl[:, t, :],
                                         start=False, stop=False)
                        nc.tensor.matmul(out=vacc[mt][:, :], lhsT=lh, rhs=wv[:, t, :],
                                         start=first, stop=last)
                        # ll-stationary group
                        nc.tensor.matmul(out=qacc[mt][:, :], lhsT=ll, rhs=wqh[:, t, :],
                                         start=False, stop=last)
                        nc.tensor.matmul(out=kacc[mt][:, :], lhsT=ll, rhs=wkh[:, t, :],
                                         start=False, stop=last)

            # ---- k: split hi/lo f16 and store ----
            for mt in range(2):
                kf = sp.tile([128, OS], f32, tag=f"kf{mt}")
                nc.scalar.copy(out=kf[:, :], in_=kacc[mt][:, :])
                kh16 = sp.tile([128, OS], f16, tag=f"kh{mt}")
                nc.scalar.copy(out=kh16[:, :], in_=kf[:, :])
                kh32 = sp.tile([128, OS], f32, tag=f"kh32{mt}")
                nc.scalar.copy(out=kh32[:, :], in_=kh16[:, :])
                kl16 = sp.tile([128, OS], f16, tag=f"kl{mt}")
                nc.vector.tensor_sub(out=kl16[:, :], in0=kf[:, :], in1=kh32[:, :])
                nc.sync.dma_start(out=KH[mt * 128:(mt + 1) * 128, :], in_=kh16[:, :])
                nc.sync.dma_start(out=KL[mt * 128:(mt + 1) * 128, :], in_=kl16[:, :])

            # ---- v to f16; u tables: u[b,k] = (v_b^T @ cw_k) as 4 M-tiles ----
            vsb = {}
            for mt in range(2):
                v16 = sp.tile([128, OS], f16, tag=f"vsb{mt}")
                nc.scalar.copy(out=v16[:, :], in_=vacc[mt][:, :])
                vsb[mt] = v16
            for b in range(B):
                off = (b % 2) * C
                vt = vsb[b // 2][off:off + C, :]  # [64, 512] f16
                for k in range(K):
                    u16 = sp.tile([128, 4, C], f16, tag="u16")
                    for m in range(4):
                        pt = pu.tile([128, C], f32, tag="pu")
                        nc.tensor.matmul(out=pt[:, :],
                                         lhsT=vt[:, m * 128:(m + 1) * 128],
                                         rhs=cw[off:off + C, k * C:(k + 1) * C],
                                         start=True, stop=True)
                        nc.scalar.copy(out=u16[:, m, :], in_=pt[:, :])
                    nc.sync.dma_start(out=UO[b, k], in_=u16[:, :, :])

            # ---- qn = q / ||q||_col, emitted as hi/lo f16 ----
            for b in range(B):
                off = (b % 2) * C
                q_b = qacc[b // 2][off:off + C, :]  # [64, 512] f32 psum
                qf = sp.tile([128, OS], f32, tag="qf")
                nc.scalar.copy(out=qf[off:off + C, :], in_=q_b)
                sq = sp.tile([128, OS], f32, tag="sq")
                nc.scalar.square(out=sq[off:off + C, :], in_=qf[off:off + C, :])
                pscr = pu.tile([128, OS], f32, tag="pscr", name="pscr")
                nc.tensor.matmul(out=pscr[0:1, :], lhsT=ones_r[off:off + C, :],
                                 rhs=sq[off:off + C, :], start=True, stop=True)
                nrm = sp.tile([1, OS], f32, tag="nrm")
                nc.scalar.sqrt(out=nrm[:, :], in_=pscr[0:1, :])
                rcp = sp.tile([1, OS], f32, tag="rcp")
                nc.vector.reciprocal(out=rcp[:, :], in_=nrm[:, :])
                pb = pu.tile([128, OS], f32, tag="pscr", name="pb")
                nc.tensor.matmul(out=pb[off:off + C, :], lhsT=ones_b[:, :],
                                 rhs=rcp[:, :], start=True, stop=True)
                bc = sp.tile([128, OS], f32, tag="bc")
                nc.scalar.copy(out=bc[off:off + C, :], in_=pb[off:off + C, :])
                qn = sp.tile([128, OS], f32, tag="qn")
                nc.vector.tensor_mul(out=qn[off:off + C, :], in0=qf[off:off + C, :],
                                     in1=bc[off:off + C, :])
                qh16 = sp.tile([128, OS], f16, tag="qh16")
                nc.scalar.copy(out=qh16[off:off + C, :], in_=qn[off:off + C, :])
                qh32 = sp.tile([128, OS], f32, tag="qh32")
                nc.scalar.copy(out=qh32[off:off + C, :], in_=qh16[off:off + C, :])
                ql16 = sp.tile([128, OS], f16, tag="ql16")
                nc.vector.tensor_sub(out=ql16[off:off + C, :],
                                     in0=qn[off:off + C, :], in1=qh32[off:off + C, :])
                nc.sync.dma_start(out=QNH[b * C:(b + 1) * C, :], in_=qh16[off:off + C, :])
                nc.sync.dma_start(out=QNL[b * C:(b + 1) * C, :], in_=ql16[off:off + C, :])
    nc.compile()
    return nc


def _build_l2():
    """Phase 2, SPMD core j: rows t in [j*512,(j+1)*512) for all batches.
    Exact sim via fp16 split-3 matmuls in 4 psum chunks of 1024; per-chunk
    exact top-8 via max/max_index on the f32 copy; merge 32 candidates via
    max8 + find_index + one-hot decode; gather-sum u tables -> yT;
    partial out = yT^T @ WoT slice."""
    nc = bacc.Bacc("TRN2", target_bir_lowering=False, debug=False, num_devices=NCORES)
    QNH = nc.dram_tensor("qnh", [BC, T], f16, kind="ExternalInput").ap()
    QNL = nc.dram_tensor("qnl", [BC, T], f16, kind="ExternalInput").ap()
    KHJ = nc.dram_tensor("khj", [BC, OS], f16, kind="ExternalInput").ap()
    KLJ = nc.dram_tensor("klj", [BC, OS], f16, kind="ExternalInput").ap()
    WOT = nc.dram_tensor("wot", [OS, T], f16, kind="ExternalInput").ap()
    IOTA = nc.dram_tensor("iota32", [128, 32], u32, kind="ExternalInput").ap()
    UT = [[nc.dram_tensor(f"ut{b}_{k}", [T, C], f16, kind="ExternalInput").ap()
           for k in range(K)] for b in range(B)]
    OUT = nc.dram_tensor("out_o", [2, 128, T], f32, kind="ExternalOutput").ap()

    NCH = 4          # sim chunks per block
    CW_ = T // NCH   # 1024 columns per chunk

    with tile.TileContext(nc) as tc:
        with tc.tile_pool(name="qp", bufs=1) as qp, \
             tc.tile_pool(name="wp", bufs=1) as wp, \
             tc.tile_pool(name="sp", bufs=4) as sp, \
             tc.tile_pool(name="simp", bufs=6) as simp, \
             tc.tile_pool(name="yp", bufs=1) as yp, \
             tc.tile_pool(name="obp", bufs=2) as obp, \
             tc.tile_pool(name="pp", bufs=4, space="PSUM") as pp, \
             tc.tile_pool(name="po", bufs=2, space="PSUM") as po:
            kh, kl = {}, {}
            for mt in range(2):
                h16 = qp.tile([128, OS], f16, tag=f"kh{mt}")
                nc.sync.dma_start(out=h16[:, :], in_=KHJ[mt * 128:(mt + 1) * 128, :])
                l16 = qp.tile([128, OS], f16, tag=f"kl{mt}")
                nc.sync.dma_start(out=l16[:, :], in_=KLJ[mt * 128:(mt + 1) * 128, :])
                kh[mt], kl[mt] = h16, l16
            qh, ql = {}, {}
            for mt in range(2):
                h16 = qp.tile([128, T], f16, tag=f"qh{mt}")
                l16 = qp.tile([128, T], f16, tag=f"ql{mt}")
                for cc in range(4):
                    cs = slice(cc * 1024, (cc + 1) * 1024)
                    nc.sync.dma_start(out=h16[:, cs], in_=QNH[mt * 128:(mt + 1) * 128, cs])
                    nc.sync.dma_start(out=l16[:, cs], in_=QNL[mt * 128:(mt + 1) * 128, cs])
                qh[mt], ql[mt] = h16, l16
            wot = {}
            iota = qp.tile([128, 32], u32, tag="iota")
            nc.sync.dma_start(out=iota[:, :], in_=IOTA[:, :])

            wu = qp.tile([128, 512], f16, tag="wu")
            nc.vector.memset(wu[:, :], 0.0)
            for _w in range(8):
                pw = po.tile([128, CW_], f32, tag="po", name=f"warm{_w}")
                nc.tensor.matmul(out=pw[:, :512], lhsT=wu[:, :128], rhs=wu[:, :],
                                 start=True, stop=True)

            ytp = {}  # (pair, kt) -> [128, 128] f16 lhsT tiles for the out matmul
            for pair in range(2):
                for kt in range(4):
                    yt_tile = yp.tile([128, 128], f16, tag=f"yt{pair}{kt}")
                    ytp[(pair, kt)] = yt_tile

            for b in range(B):
                khb = kh[b // 2][(b % 2) * C:(b % 2) * C + C, :]   # [64, 512] f16
                klb = kl[b // 2][(b % 2) * C:(b % 2) * C + C, :]
                qhb = qh[b // 2][(b % 2) * C:(b % 2) * C + C, :]   # [64, 4096] f16
                qlb = ql[b // 2][(b % 2) * C:(b % 2) * C + C, :]
                for i in range(4):  # 128-token blocks of this core's slice
                    lh = khb[:, i * 128:(i + 1) * 128]
                    ll = klb[:, i * 128:(i + 1) * 128]
                    simf = simp.tile([128, T], f32, tag="simf")
                    for ch in range(NCH):
                        for half in range(2):
                            c0 = ch * CW_ + half * 512
                            rh = qhb[:, c0:c0 + 512]
                            rl = qlb[:, c0:c0 + 512]
                            ps = pp.tile([128, 512], f32, tag="ps")
                            nc.tensor.matmul(out=ps[:, :], lhsT=lh, rhs=rh,
                                             start=True, stop=False)
                            nc.tensor.matmul(out=ps[:, :], lhsT=ll, rhs=rh,
                                             start=False, stop=False)
                            nc.tensor.matmul(out=ps[:, :], lhsT=lh, rhs=rl,
                                             start=False, stop=True)
                            nc.scalar.copy(out=simf[:, c0:c0 + 512], in_=ps[:, :])
                    # one full-row top-8 scan + one full-row index scan
                    g8 = sp.tile([128, 8], f32, tag="g8")
                    nc.vector.max(out=g8[:, :], in_=simf[:, :])
                    idx8 = sp.tile([128, 8], u32, tag="idx8")
                    nc.vector.max_index(out=idx8[:, :], in_max=g8[:, :],
                                        in_values=simf[:, :])
                    gth = sp.tile([128, K, C], f16, tag="gth")
                    for k in range(K):
                        nc.gpsimd.indirect_dma_start(
                            out=gth[:, k, :], out_offset=None,
                            in_=UT[b][k][:, :],
                            in_offset=bass.IndirectOffsetOnAxis(ap=idx8[:, k:k + 1], axis=0))
                    t0 = sp.tile([128, C], f16, tag="t0")
                    t1 = sp.tile([128, C], f16, tag="t1")
                    nc.gpsimd.tensor_add(out=t0[:, :], in0=gth[:, 0, :], in1=gth[:, 1, :])
                    nc.gpsimd.tensor_add(out=t1[:, :], in0=gth[:, 2, :], in1=gth[:, 3, :])
                    dst = ytp[(b // 2, i)][:, (b % 2) * C:(b % 2) * C + C]
                    nc.gpsimd.tensor_add(out=dst, in0=t0[:, :], in1=t1[:, :])

                # emit the pair's out-projection as soon as both batches done
                if b % 2 == 1:
                    pair = b // 2
                    if pair == 1:
                        # keep PE warm through the tail (no-dep matmuls)
                        for _w in range(10):
                            pw = pp.tile([128, 512], f32, tag="ps")
                            nc.tensor.matmul(out=pw[:, :], lhsT=wu[:, :128],
                                             rhs=wu[:, :], start=True, stop=True)
                    if not wot:
                        for kt in range(4):
                            w = wp.tile([128, T], f16, tag=f"wot{kt}", name=f"wot{kt}")
                            nc.sync.dma_start(out=w[:, :], in_=WOT[kt * 128:(kt + 1) * 128, :])
                            wot[kt] = w
                    ob = obp.tile([128, T], f32, tag="ob")
                    for ch in range(NCH):
                        ps = po.tile([128, CW_], f32, tag="po")
                        for half in range(2):
                            c0 = ch * CW_ + half * 512
                            po_s = ps[:, half * 512:(half + 1) * 512]
                            for kt in range(4):
                                nc.tensor.matmul(out=po_s, lhsT=ytp[(pair, kt)][:, :],
                                                 rhs=wot[kt][:, c0:c0 + 512],
                                                 start=(kt == 0), stop=(kt == 3))
                        nc.scalar.copy(out=ob[:, ch * CW_:(ch + 1) * CW_], in_=ps[:, :])
                        nc.sync.dma_start(out=OUT[pair, :, ch * CW_:(ch + 1) * CW_],
                                          in_=ob[:, ch * CW_:(ch + 1) * CW_])


    nc.compile()
    return nc


def _split16(a):
    h = a.astype(np.float16)
    l = (a - h.astype(np.float32)).astype(np.float16)
    return h, l


def kernel(x, Wq, Wk, Wv, Wo, conv_w, conv_b):
    x = np.asarray(x, np.float32)
    Wq = np.asarray(Wq, np.float32); Wk = np.asarray(Wk, np.float32)
    Wv = np.asarray(Wv, np.float32); Wo = np.asarray(Wo, np.float32)
    conv_w = np.asarray(conv_w, np.float32); conv_b = np.asarray(conv_b, np.float32)

    LAST_EXEC_NS.clear(); LAST_TRACE_DIRS.clear()
    if "l1" not in _cache:
        _cache["l1"] = _build_l1()
    if "l2" not in _cache:
        _cache["l2"] = _build_l2()

    def _sw(a):
        # [T, W] -> [128, T//128, W] with [p, kt, w] = a[kt*128+p, w]
        return np.ascontiguousarray(a.reshape(T // 128, 128, -1).transpose(1, 0, 2))

    xT = np.ascontiguousarray(x.transpose(2, 0, 1).reshape(T, BC))  # [t, b*64+c]
    xh, xl = _split16(xT)
    xh, xl = _sw(xh), _sw(xl)
    WqT, WkT = Wq.T, Wk.T
    WvT16 = np.ascontiguousarray(Wv.T).astype(np.float16)
    cw1 = np.ascontiguousarray(conv_w.transpose(1, 2, 0).reshape(C, K * C)).astype(np.float16)
    cw = np.concatenate([cw1, cw1], axis=0)
    # cw[ci, k*64+co] = conv_w[co, ci, k]

    in_maps = []
    for j in range(NCORES):
        sl = slice(j * OS, (j + 1) * OS)
        wqh, wql = _split16(np.ascontiguousarray(WqT[:, sl]))
        wkh, wkl = _split16(np.ascontiguousarray(WkT[:, sl]))
        in_maps.append({"xh": xh, "xl": xl,
                        "wqh": _sw(wqh), "wql": _sw(wql),
                        "wkh": _sw(wkh), "wkl": _sw(wkl),
                        "wv": _sw(np.ascontiguousarray(WvT16[:, sl])), "cw": cw})
    r1 = _run(_cache["l1"], in_maps)

    qnh = np.concatenate([r1.results[j]["qnh_o"] for j in range(NCORES)], axis=1)
    qnl = np.concatenate([r1.results[j]["qnl_o"] for j in range(NCORES)], axis=1)
    ut = {}
    for b in range(B):
        for k in range(K):
            ut[(b, k)] = np.ascontiguousarray(np.concatenate(
                [r1.results[j]["u_o"][b, k].transpose(1, 0, 2).reshape(OS, C)
                 for j in range(NCORES)], axis=0))

    iota32 = np.broadcast_to(np.arange(32, dtype=np.uint32), (128, 32)).copy()
    in_maps2 = []
    for j in range(NCORES):
        m = {"qnh": qnh, "qnl": qnl,
             "khj": r1.results[j]["kh_o"], "klj": r1.results[j]["kl_o"],
             "wot": np.ascontiguousarray(Wo.T[j * OS:(j + 1) * OS, :]).astype(np.float16),
             "iota32": iota32}
        for b in range(B):
            for k in range(K):
                m[f"ut{b}_{k}"] = ut[(b, k)]
        in_maps2.append(m)
    r2 = _run(_cache["l2"], in_maps2)

    out = np.zeros((B, C, T), np.float32)
    for j in range(NCORES):
        oo = r2.results[j]["out_o"]  # [2, 128, T]
        for b in range(B):
            out[b] += oo[b // 2, (b % 2) * C:(b % 2) * C + C, :]
    bias = conv_b[:, None] * Wo.sum(axis=1)[None, :]  # [64, 4096]
    out += bias[None, :, :]
    return out
